# revision 1
# baseline (speedup 1.0000x reference)
"""Trainium2 Bass kernel for nn_DeepseekLayer (dense transformer layer).

Sharding (8 cores): Megatron-style TP, bf16 datapath (fp32 PSUM accum).
  - qkv head-sharded (2 heads/core) over full S; x resident in SBUF (bf16).
  - rmsnorm1 folded into rope tables / V-scale (scaling commutes with the
    linear projections); rstd computed on device.
  - attention: transposed-softmax layout (scores [sk, sq]), exp on ACT in
    [128,1024] chunks, fast-reciprocal normalize, per-head AllToAll (bf16)
    switches attention output to token shards so o_proj needs no all-reduce.
  - o_proj + residual + rmsnorm2: token-sharded (256 tokens/core), fp32
    residual.
  - MLP: AllGather hidden (bf16, 2 token-half chunks pipelined under MLP)
    -> tensor-parallel gate/up/down (1024 ff dims/core, weights streamed)
    -> per-half bf16 ReduceScatter overlapped with the other half -> local
    residual add -> host gathers token shards.
"""
import numpy as np
from contextlib import ExitStack

import ml_dtypes
from concourse import bacc
import concourse.tile as tile
import concourse.mybir as mybir
from concourse.bass_utils import run_bass_kernel_spmd

F32 = mybir.dt.float32
BF = mybir.dt.bfloat16
AF = mybir.ActivationFunctionType
OP = mybir.AluOpType

H = 2048          # hidden
NH = 16           # heads
HD = 128          # head dim
MLP = 8192
S = 2048          # sequence
B = 1
EPS = 1e-6
NC = 8            # cores
HPC = NH // NC    # heads per core = 2
EH = HPC * HD     # qkv out dims per core = 256
MSH = MLP // NC   # mlp dims per core = 1024
SSH = S // NC     # tokens per shard = 256
RG = [list(range(NC))]
DT = H // 128     # 16 d-tiles
MT = MSH // 128   # 8 m-tiles per core
BF_NP = ml_dtypes.bfloat16

_LDW_PATCHED = False


def _enable_ldw_opt():
    """Compile our NEFF with walrus --enable-ldw-opt=true (elides redundant
    LDWEIGHTS; concourse's default is false)."""
    global _LDW_PATCHED
    if _LDW_PATCHED:
        return
    import concourse.bass_utils as _bu
    _orig = _bu.run_command

    def _patched(argv, **kw):
        argv = ["--enable-ldw-opt=true" if a == "--enable-ldw-opt=false" else a
                for a in argv]
        return _orig(argv, **kw)

    _bu.run_command = _patched
    _LDW_PATCHED = True


def _build_program():
    # NOTE: walrus --enable-ldw-opt rejects explicit InstLdweights (emitted for
    # bf16 matmuls); bf16 stationaries get FWL instead, so keep the default.
    nc = bacc.Bacc(trn_type="TRN2", target_bir_lowering=False, debug=False,
                   num_devices=NC)

    def inp(name, shape, dt):
        return nc.dram_tensor(name, shape, dt, kind="ExternalInput").ap()

    xT = inp("xT", [H, S], BF)                  # x.T (feature-major), bf16
    xTrs = inp("xTrs", [H, SSH], F32)           # this core's token-shard, f32
    cosT = inp("cosT", [HD, S], F32)
    sinTs = inp("sinTs", [HD, S], F32)          # sin, rows 0:63 pre-negated
    wqkvT = inp("wqkvT", [H, 6 * 128], BF)      # cols: q0,q1,k0,k1,v0,v1
    woTt = inp("woTt", [DT, 128, DT, 128], BF)  # wo.T tiled [dt, p, et, c]
    wgTt = inp("wgTt", [MT, 128, DT, 128], BF)  # (wg*n2w).T shard [mt, p, dt, c]
    wuTt = inp("wuTt", [MT, 128, DT, 128], BF)
    wdTt = inp("wdTt", [DT, 128, MT, 128], BF)  # wd shard.T tiled [dt, p, mt, c]
    out_sh = nc.dram_tensor("out_sh", [H, SSH], F32, kind="ExternalOutput").ap()

    with tile.TileContext(nc) as tc, ExitStack() as top:
        dram = top.enter_context(tc.tile_pool(name="dram", bufs=1, space="DRAM"))
        per = top.enter_context(tc.tile_pool(name="per", bufs=1))
        ones_f = per.tile([128, 1], F32)
        nc.gpsimd.memset(ones_f[:], 1.0)
        ones_b = per.tile([128, 1], BF)
        nc.vector.tensor_copy(ones_b[:], ones_f[:])
        eps1 = per.tile([1, 1], F32)
        nc.gpsimd.memset(eps1[:], EPS)
        from concourse.masks import make_identity
        ident_f = per.tile([128, 128], F32)
        make_identity(nc, ident_f[:])
        ident_b = per.tile([128, 128], BF)
        nc.vector.tensor_copy(ident_b[:], ident_f[:])

        # ---- persistent SBUF: fp32 residual + attention I/O per head ----
        res_ctx = ExitStack()
        res_pool = res_ctx.enter_context(tc.tile_pool(name="res", bufs=1))
        res1 = [res_pool.tile([128, SSH], F32, name=f"res1_{dt}") for dt in range(DT)]
        xr = res_pool.tile([128, DT, SSH], F32, name="xr")

        qk_ctx = ExitStack()
        qk = qk_ctx.enter_context(tc.tile_pool(name="qk", bufs=1))
        qr = [qk.tile([128, S], BF, name=f"qr{h}") for h in range(HPC)]
        kr = [qk.tile([128, S], BF, name=f"kr{h}") for h in range(HPC)]
        V_sb = qk.tile([128, S // 128, EH], BF, name="V_sb")
        att = [qk.tile([128, S], BF, name=f"att{h}") for h in range(HPC)]

        a2a_in = [dram.tile([NC, 128, SSH], BF, name=f"a2a_in{h}") for h in range(HPC)]
        a2a_out = [dram.tile([NC, 128, SSH], BF, name=f"a2a_out{h}") for h in range(HPC)]

        # x / tables / qkv weights: live through P1+P2
        x_ctx = ExitStack()
        xp = x_ctx.enter_context(tc.tile_pool(name="xp", bufs=1))
        x_sb = xp.tile([128, DT, S], BF, name="x_sb")
        wqkv_sb = xp.tile([128, DT, 6 * 128], BF, name="wqkv_sb")
        cs_c = xp.tile([HD, S], F32, name="cs_c")     # cos * rstd
        cs_s = xp.tile([HD, S], F32, name="cs_s")     # (+-)sin * rstd
        rstd_bc = xp.tile([128, S], F32, name="rstd_bc")

        def rope(pool, dst, ps, c0):
            # dst[:, c0:c0+1024] = rotate-half rope on ps, rstd folded in cs_*
            mc = pool.tile([128, 1024], F32, tag="mc", bufs=1)
            msw = pool.tile([128, 1024], F32, tag="msw", bufs=1)
            nc.vector.tensor_tensor(out=mc[:], in0=ps[:],
                                    in1=cs_c[:, c0:c0 + 1024], op=OP.mult)
            nc.vector.tensor_tensor(out=msw[0:64, :], in0=ps[64:128, :],
                                    in1=cs_s[0:64, c0:c0 + 1024], op=OP.mult)
            nc.vector.tensor_tensor(out=msw[64:128, :], in0=ps[0:64, :],
                                    in1=cs_s[64:128, c0:c0 + 1024], op=OP.mult)
            nc.vector.tensor_tensor(out=dst[:, c0:c0 + 1024], in0=mc[:],
                                    in1=msw[:], op=OP.add)

        def qkv_oc(pool, pspool, h, kind, tag):
            oc = {"q": 0, "k": 2, "v": 4}[kind] + h
            for half in range(2):
                c0 = half * 1024
                ps = pspool.tile([128, 1024], F32, tag=tag, name=f"{kind}{h}_{half}",
                                 bufs=2)
                for dt in range(DT):
                    for i in range(2):
                        nc.tensor.matmul(
                            ps[:, i * 512:(i + 1) * 512],
                            wqkv_sb[:, dt, oc * 128:(oc + 1) * 128],
                            x_sb[:, dt, c0 + i * 512:c0 + (i + 1) * 512],
                            start=(dt == 0), stop=(dt == DT - 1))
                if kind == "q":
                    rope(pool, qr[h], ps[:], c0)
                elif kind == "k":
                    rope(pool, kr[h], ps[:], c0)
                else:
                    vsc = pool.tile([128, 1024], BF, tag="vsc", bufs=2)
                    nc.vector.tensor_tensor(out=vsc[:], in0=ps[:],
                                            in1=rstd_bc[:, c0:c0 + 1024], op=OP.mult)
                    for j in range(8):
                        kt = half * 8 + j
                        tr = pspool.tile([128, 128], BF, tag=tag,
                                         name=f"tr{h}{kt}", bufs=2)
                        nc.tensor.transpose(tr[:], vsc[:, j * 128:(j + 1) * 128],
                                            ident_b[:])
                        nc.vector.tensor_copy(V_sb[:, kt, h * 128:(h + 1) * 128], tr[:])

        # ============ P1: x load + rmsnorm1 stats + qkv head0 ============
        p1_ctx = ExitStack()
        s1 = p1_ctx.enter_context(tc.tile_pool(name="s1", bufs=1))
        ps1 = p1_ctx.enter_context(tc.tile_pool(name="ps1", bufs=1, space="PSUM"))

        ss_ps = ps1.tile([1, S], F32, tag="stat", name="ss_ps", bufs=1)
        for dt in range(DT):
            nc.sync.dma_start(x_sb[:, dt, :], xT[dt * 128:(dt + 1) * 128, :])
            nc.sync.dma_start(wqkv_sb[:, dt, :], wqkvT[dt * 128:(dt + 1) * 128, :])
            if dt == 2:
                nc.sync.dma_start(cs_c[:], cosT)
                nc.sync.dma_start(cs_s[:], sinTs)
            for i in range(2):
                sq = s1.tile([128, 1024], BF, tag="sq", bufs=2)
                nc.vector.tensor_tensor(out=sq[:],
                                        in0=x_sb[:, dt, i * 1024:(i + 1) * 1024],
                                        in1=x_sb[:, dt, i * 1024:(i + 1) * 1024],
                                        op=OP.mult)
                for j in range(2):
                    c = i * 1024 + j * 512
                    nc.tensor.matmul(ss_ps[:, c:c + 512], ones_b[:],
                                     sq[:, j * 512:(j + 1) * 512],
                                     start=(dt == 0), stop=(dt == DT - 1))
        # rstd = 1/sqrt(mean+eps); broadcast; fold into rope tables
        for i in range(4):
            sdc = s1.tile([1, 512], F32, tag="sdc", bufs=2)
            nc.scalar.activation(sdc[:], ss_ps[:, i * 512:(i + 1) * 512], AF.Sqrt,
                                 bias=eps1[:], scale=1.0 / H)
            rsc = s1.tile([1, 512], F32, tag="rsc", bufs=2)
            nc.vector.reciprocal_approx_fast(out=rsc[:], in_=sdc[:])
            nc.gpsimd.partition_broadcast(rstd_bc[:, i * 512:(i + 1) * 512], rsc[:],
                                          channels=128)
        # fold rstd into the rope tables in place
        nc.vector.tensor_tensor(out=cs_c[:], in0=cs_c[:], in1=rstd_bc[:], op=OP.mult)
        nc.vector.tensor_tensor(out=cs_s[:], in0=cs_s[:], in1=rstd_bc[:], op=OP.mult)

        qkv_oc(s1, ps1, 0, "q", "qk")
        qkv_oc(s1, ps1, 0, "k", "qk")
        qkv_oc(s1, ps1, 0, "v", "qk")
        p1_ctx.close()

        # ============ P2: attention h0, qkv h1, attention h1 ============
        p2_ctx = ExitStack()
        s2 = p2_ctx.enter_context(tc.tile_pool(name="s2", bufs=1))
        ps2 = p2_ctx.enter_context(tc.tile_pool(name="ps2", bufs=1, space="PSUM"))

        def attn_head(h):
            for sc in range(2):          # sq chunks of 1024
                c0 = sc * 1024
                av = ps2.tile([128, 1024], F32, tag="av", name=f"av{h}{sc}", bufs=1)
                sm = ps2.tile([1, 1024], F32, tag="sm", name=f"sm{h}{sc}", bufs=1)
                for kt in range(DT):
                    st = ps2.tile([128, 1024], F32, tag="big", name=f"st{h}{sc}",
                                  bufs=2)
                    for i in range(2):
                        nc.tensor.matmul(st[:, i * 512:(i + 1) * 512],
                                         kr[h][:, kt * 128:(kt + 1) * 128],
                                         qr[h][:, c0 + i * 512:c0 + (i + 1) * 512],
                                         start=True, stop=True)
                    e = s2.tile([128, 1024], BF, tag="e", bufs=3)
                    nc.scalar.activation(e[:], st[:], AF.Exp)
                    for i in range(2):
                        nc.tensor.matmul(sm[:, i * 512:(i + 1) * 512], ones_b[:],
                                         e[:, i * 512:(i + 1) * 512],
                                         start=(kt == 0), stop=(kt == DT - 1))
                        nc.tensor.matmul(av[:, i * 512:(i + 1) * 512],
                                         V_sb[:, kt, h * 128:(h + 1) * 128],
                                         e[:, i * 512:(i + 1) * 512],
                                         start=(kt == 0), stop=(kt == DT - 1))
                rs = s2.tile([1, 1024], F32, tag="rs", bufs=1)
                nc.vector.reciprocal_approx_fast(out=rs[:], in_=sm[:])
                bc = s2.tile([128, 1024], F32, tag="bc", bufs=1)
                nc.gpsimd.partition_broadcast(bc[:], rs[:], channels=128)
                nc.vector.tensor_tensor(out=att[h][:, c0:c0 + 1024], in0=av[:],
                                        in1=bc[:], op=OP.mult)
            for j in range(NC):
                nc.sync.dma_start(a2a_in[h][j], att[h][:, j * SSH:(j + 1) * SSH])
            nc.gpsimd.collective_compute("AllToAll", OP.bypass,
                                         ins=[a2a_in[h][:]], outs=[a2a_out[h][:]],
                                         replica_groups=RG)

        attn_head(0)
        qkv_oc(s2, ps2, 1, "q", "big")
        qkv_oc(s2, ps2, 1, "k", "big")
        qkv_oc(s2, ps2, 1, "v", "big")
        for dt in range(DT):   # residual slice; needed from o_proj onwards
            nc.sync.dma_start(xr[:, dt, :], xTrs[dt * 128:(dt + 1) * 128, :])
        attn_head(1)
        p2_ctx.close()
        x_ctx.close()
        qk_ctx.close()

        # ============ P3: o_proj + residual + rmsnorm2 + AG ============
        h2_ctx = ExitStack()
        h2p = h2_ctx.enter_context(tc.tile_pool(name="h2p", bufs=1))
        h2 = [h2p.tile([128, SSH], BF, name=f"h2_{dt}") for dt in range(DT)]
        ag_in = [dram.tile([128, DT, 128], BF, name=f"ag_in{t}") for t in range(2)]
        ag_out = [dram.tile([NC, 128, DT, 128], BF, addr_space="Shared",
                            name=f"ag_out{t}") for t in range(2)]
        with tc.tile_pool(name="s3", bufs=1) as s3, \
             tc.tile_pool(name="ps3", bufs=1, space="PSUM") as ps3:
            attg = s3.tile([128, DT, SSH], BF, tag="attg")
            for j in range(NC):
                for h in range(HPC):
                    nc.sync.dma_start(attg[:, 2 * j + h, :], a2a_out[h][j])
            ss2_ps = ps3.tile([1, SSH], F32, tag="ss2", name="ss2_ps")
            for dt in range(DT):
                wo_t = s3.tile([128, DT, 128], BF, tag="wo", bufs=3)
                nc.sync.dma_start(wo_t[:], woTt[dt])
                o_ps = ps3.tile([128, SSH], F32, tag="o", name="o_ps", bufs=3)
                for et in range(DT):
                    nc.tensor.matmul(o_ps[:], wo_t[:, et, :], attg[:, et, :],
                                     start=(et == 0), stop=(et == DT - 1))
                nc.vector.tensor_tensor(out=res1[dt][:], in0=o_ps[:],
                                        in1=xr[:, dt, :], op=OP.add)
                sq2 = s3.tile([128, SSH], BF, tag="sq2", bufs=2)
                nc.vector.tensor_tensor(out=sq2[:], in0=res1[dt][:], in1=res1[dt][:],
                                        op=OP.mult)
                nc.tensor.matmul(ss2_ps[:], ones_b[:], sq2[:],
                                 start=(dt == 0), stop=(dt == DT - 1))
            sd2 = s3.tile([1, SSH], F32, name="sd2")
            nc.scalar.activation(sd2[:], ss2_ps[:], AF.Sqrt, bias=eps1[:],
                                 scale=1.0 / H)
            rstd2 = s3.tile([1, SSH], F32, name="rstd2")
            nc.vector.reciprocal_approx_fast(out=rstd2[:], in_=sd2[:])
            rstd2_bc = s3.tile([128, SSH], F32, name="rstd2_bc")
            nc.gpsimd.partition_broadcast(rstd2_bc[:], rstd2[:], channels=128)
            for dt in range(DT):
                nc.vector.tensor_tensor(out=h2[dt][:], in0=res1[dt][:],
                                        in1=rstd2_bc[:], op=OP.mult)
            # AG per token-half so MLP(ts0) can start while AG(ts1) flies
            for ts in range(2):
                for dt in range(DT):
                    nc.sync.dma_start(ag_in[ts][:, dt, :],
                                      h2[dt][:, ts * 128:(ts + 1) * 128])
                nc.gpsimd.collective_compute("AllGather", OP.bypass,
                                             ins=[ag_in[ts][:]], outs=[ag_out[ts][:]],
                                             replica_groups=RG)

        # ============ P4: MLP per token-half, RS chunked by d-half ============
        HDT = DT // 2
        rs_in = [[dram.tile([NC, 128, HDT, 128], BF, name=f"rs_in{t}{g}")
                  for g in range(2)] for t in range(2)]
        rs_out = [[dram.tile([128, HDT, 128], BF, name=f"rs_out{t}{g}")
                   for g in range(2)] for t in range(2)]
        with tc.tile_pool(name="s4", bufs=1) as s4, \
             tc.tile_pool(name="ps4", bufs=1, space="PSUM") as ps4:
            for ts in range(2):
                # h2g layout [p, src-core j, dt, c]; tokens within the half are
                # j-major (matches rs/out mapping below)
                h2g = s4.tile([128, NC, DT, 128], BF, tag="h2g", bufs=2,
                              name=f"h2g{ts}")
                for j in range(NC):
                    nc.sync.dma_start(h2g[:, j], ag_out[ts][j])
                act_t = s4.tile([128, MT, 1024], BF, tag="act", bufs=1,
                                name=f"act{ts}")
                for mt in range(MT):
                    wg_t = s4.tile([128, DT, 128], BF, tag="wg", bufs=2)
                    wu_t = s4.tile([128, DT, 128], BF, tag="wu", bufs=2)
                    nc.sync.dma_start(wg_t[:], wgTt[mt])
                    nc.sync.dma_start(wu_t[:], wuTt[mt])
                    g_ps = ps4.tile([128, 1024], F32, tag="g", name=f"g{ts}{mt}",
                                    bufs=2)
                    u_ps = ps4.tile([128, 1024], F32, tag="u", name=f"u{ts}{mt}",
                                    bufs=2)
                    for dt in range(DT):
                        for i in range(2):
                            nc.tensor.matmul(
                                g_ps[:, i * 512:(i + 1) * 512], wg_t[:, dt, :],
                                h2g[:, 4 * i:4 * (i + 1), dt, :],
                                start=(dt == 0), stop=(dt == DT - 1))
                    for dt in range(DT):
                        for i in range(2):
                            nc.tensor.matmul(
                                u_ps[:, i * 512:(i + 1) * 512], wu_t[:, dt, :],
                                h2g[:, 4 * i:4 * (i + 1), dt, :],
                                start=(dt == 0), stop=(dt == DT - 1))
                    gs = s4.tile([128, 1024], BF, tag="gs", bufs=2)
                    nc.scalar.activation(gs[:], g_ps[:], AF.Sigmoid)
                    nc.vector.tensor_tensor(out=act_t[:, mt, :], in0=u_ps[:],
                                            in1=gs[:], op=OP.mult)
                # down: contraction over mt; RS fired per d-half
                dn_all = s4.tile([128, DT, 1024], BF, tag="dn", bufs=1,
                                 name=f"dn{ts}")
                for grp in range(2):
                    for dt in range(grp * HDT, (grp + 1) * HDT):
                        wd_t = s4.tile([128, MT, 128], BF, tag="wd", bufs=3)
                        nc.sync.dma_start(wd_t[:], wdTt[dt])
                        d_ps = ps4.tile([128, 1024], F32,
                                        tag=("g" if dt % 2 == 0 else "u"),
                                        name=f"d{ts}{dt}", bufs=2)
                        for mt in range(MT):
                            for i in range(2):
                                nc.tensor.matmul(
                                    d_ps[:, i * 512:(i + 1) * 512], wd_t[:, mt, :],
                                    act_t[:, mt, i * 512:(i + 1) * 512],
                                    start=(mt == 0), stop=(mt == MT - 1))
                        nc.vector.tensor_copy(dn_all[:, dt, :], d_ps[:])
                    for j in range(NC):
                        nc.sync.dma_start(
                            rs_in[ts][grp][j],
                            dn_all[:, grp * HDT:(grp + 1) * HDT,
                                   j * 128:(j + 1) * 128])
                    nc.gpsimd.collective_compute(
                        "ReduceScatter", OP.add, ins=[rs_in[ts][grp][:]],
                        outs=[rs_out[ts][grp][:]], replica_groups=RG)
                # finalize this half as soon as its RS lands (overlaps next work)
                for grp in range(2):
                    rsb = s4.tile([128, HDT, 128], BF, tag="rsb", bufs=2)
                    nc.sync.dma_start(rsb[:], rs_out[ts][grp][:])
                    for k in range(HDT):
                        dt = grp * HDT + k
                        fin = s4.tile([128, 128], F32, tag="fin", bufs=4)
                        nc.vector.tensor_tensor(
                            out=fin[:], in0=rsb[:, k, :],
                            in1=res1[dt][:, ts * 128:(ts + 1) * 128], op=OP.add)
                        nc.sync.dma_start(out_sh[dt * 128:(dt + 1) * 128,
                                                 ts * 128:(ts + 1) * 128], fin[:])

        h2_ctx.close()
        res_ctx.close()

    nc.compile()
    return nc


_PROG = None


def _get_program():
    global _PROG
    if _PROG is None:
        _PROG = _build_program()
    return _PROG


def _prep_inputs(x, norm1_w, wq, wk, wv, wo, norm2_w, w_gate, w_up, w_down, cos, sin):
    x = np.asarray(x, dtype=np.float32)
    xT = np.ascontiguousarray(x.reshape(S, H).T)                       # [H, S]
    xT_bf = xT.astype(BF_NP)
    cosT = np.ascontiguousarray(np.asarray(cos, np.float32).T)         # [HD, S]
    sinT = np.ascontiguousarray(np.asarray(sin, np.float32).T)
    sinTs = sinT.copy()
    sinTs[0:HD // 2] = -sinTs[0:HD // 2]       # rotate_half sign for lo rows
    n1 = np.asarray(norm1_w, np.float32)
    n2 = np.asarray(norm2_w, np.float32)
    wq = np.asarray(wq, np.float32) * n1[None, :] / np.sqrt(np.float32(HD))
    wk = np.asarray(wk, np.float32) * n1[None, :]
    wv = np.asarray(wv, np.float32) * n1[None, :]
    wg = np.asarray(w_gate, np.float32) * n2[None, :]
    wu = np.asarray(w_up, np.float32) * n2[None, :]
    wo = np.asarray(wo, np.float32)
    wd = np.asarray(w_down, np.float32)

    woT = wo.T.astype(BF_NP)                                           # [e=H, d=H]
    woTt = np.ascontiguousarray(
        woT.reshape(DT, 128, DT, 128).transpose(2, 1, 0, 3))           # [dt, p, et, c]

    in_maps = []
    for c in range(NC):
        e0 = c * EH
        m0 = c * MSH
        wqkv = np.concatenate([wq[e0:e0 + EH, :], wk[e0:e0 + EH, :],
                               wv[e0:e0 + EH, :]], axis=0)             # [768, H]
        wqkvT = np.ascontiguousarray(wqkv.T).astype(BF_NP)             # [H, 768]
        wgT = wg[m0:m0 + MSH, :].T.astype(BF_NP)                       # [H, MSH]
        wuT = wu[m0:m0 + MSH, :].T.astype(BF_NP)
        wdT = wd[:, m0:m0 + MSH].T.astype(BF_NP)                       # [MSH, H]
        in_maps.append({
            "xT": xT_bf,
            "xTrs": np.ascontiguousarray(xT[:, c * SSH:(c + 1) * SSH]),
            "cosT": cosT, "sinTs": sinTs,
            "wqkvT": wqkvT,
            "woTt": woTt,
            "wgTt": np.ascontiguousarray(
                wgT.reshape(DT, 128, MT, 128).transpose(2, 1, 0, 3)),  # [mt,p,dt,c]
            "wuTt": np.ascontiguousarray(
                wuT.reshape(DT, 128, MT, 128).transpose(2, 1, 0, 3)),
            "wdTt": np.ascontiguousarray(
                wdT.reshape(MT, 128, DT, 128).transpose(2, 1, 0, 3)),  # [dt,p,mt,c]
        })
    return in_maps


def kernel(x, norm1_w, wq, wk, wv, wo, norm2_w, w_gate, w_up, w_down, cos, sin,
           _want_results=False):
    in_maps = _prep_inputs(x, norm1_w, wq, wk, wv, wo, norm2_w,
                           w_gate, w_up, w_down, cos, sin)
    prog = _get_program()
    res = run_bass_kernel_spmd(prog, in_maps, list(range(NC)))
    out = np.empty((B, S, H), dtype=np.float32)
    for c in range(NC):
        out[0, c * SSH:(c + 1) * SSH, :] = res.results[c]["out_sh"].T
    if _want_results:
        return out, res
    return out



# revision 7
# speedup vs baseline: 1.1331x; 1.1331x over previous
"""Trainium2 Bass kernel for nn_DeepseekLayer (dense transformer layer).

Sharding (8 cores): Megatron-style TP, fp8(e4m3)+bf16 datapath, fp32 PSUM.
  - qkv head-sharded (2 heads/core) over full S; x resident in SBUF as fp8
    (x*16); qkv/o/gate weights fp8 (*2^10), matmuls in DoubleRow perf mode
    (2 k-subtiles of 128 per instruction, ~1.5x bf16 throughput).
  - rmsnorm1 folded into rope tables / V-scale; rstd computed on device.
  - attention: transposed-softmax layout (scores [sk, sq]) in bf16,
    exp(score-4) written as fp8 by ACT, softmax sums + probs@V as fp8
    DoubleRow matmuls, fast-reciprocal normalize, per-head AllToAll (fp8,
    att*16) switches attention output to token shards.
  - o_proj fp8 DoubleRow (wo pre-permuted head-major) + residual +
    rmsnorm2: token-sharded (256 tokens/core), fp32 residual.
  - MLP: dual AllGather of hidden per token-half (fp8 h2*16 for gate,
    bf16 for up) -> gate fp8 DoubleRow, up/down bf16 (fp8 there would
    break the 2e-2 error budget) -> chunked bf16 ReduceScatter (last
    chunk small to shrink the exposed tail) -> local residual add.
"""
import numpy as np
from contextlib import ExitStack

import ml_dtypes
from concourse import bacc
import concourse.tile as tile
import concourse.mybir as mybir
from concourse.bass_utils import run_bass_kernel_spmd

F32 = mybir.dt.float32
BF = mybir.dt.bfloat16
F8 = mybir.dt.float8e4
AF = mybir.ActivationFunctionType
OP = mybir.AluOpType
DR = mybir.MatmulPerfMode.DoubleRow

H = 2048          # hidden
NH = 16           # heads
HD = 128          # head dim
MLP = 8192
S = 2048          # sequence
B = 1
EPS = 1e-6
NC = 8            # cores
HPC = NH // NC    # heads per core = 2
EH = HPC * HD     # qkv out dims per core = 256
MSH = MLP // NC   # mlp dims per core = 1024
SSH = S // NC     # tokens per shard = 256
RG = [list(range(NC))]
DT = H // 128     # 16 d-tiles
DTP = DT // 2     # 8 d-tile pairs
MT = MSH // 128   # 8 m-tiles per core
BF_NP = ml_dtypes.bfloat16
F8_NP = ml_dtypes.float8_e4m3   # TRN fp8e4: max +-240, like IEEE e4m3

SX = 16.0         # x / activation fp8 scale
SW = 1024.0       # weight fp8 scale (2^10)
ECONST = 4.0      # exp(score - ECONST); cancels in softmax
SO = SW * SX      # o_ps scale (wo*SW applied to att*16 -> /16 net... see below)


def _q8(a, scale):
    return np.clip(np.asarray(a, np.float32) * scale, -240.0, 240.0).astype(F8_NP)


def _build_program():
    nc = bacc.Bacc(trn_type="TRN2", target_bir_lowering=False, debug=False,
                   num_devices=NC)

    def inp(name, shape, dt):
        return nc.dram_tensor(name, shape, dt, kind="ExternalInput").ap()

    x8T = inp("x8T", [H, S], F8)                # x.T * 16, fp8
    xTrs = inp("xTrs", [H, SSH], F32)           # this core's token-shard, f32
    cosT = inp("cosT", [HD, S], F32)            # cos.T / (SW*SX)
    sinTs = inp("sinTs", [HD, S], F32)          # sin.T / (SW*SX), rows 0:63 neg
    wqkvT8 = inp("wqkvT8", [H, 6 * 128], F8)    # cols: q0,q1,k0,k1,v0,v1 (*SW)
    woTt8 = inp("woTt8", [DT, 128, DT, 128], F8)  # wo.T [dt, p, et', c] head-major et'
    wgTt8 = inp("wgTt8", [MT, 128, DT, 128], F8)  # (wg*n2).T shard *SW
    wuTt = inp("wuTt", [MT, 128, DT, 128], BF)
    wdTt = inp("wdTt", [DT, 128, MT, 128], BF)
    out_sh = nc.dram_tensor("out_sh", [H, SSH], F32, kind="ExternalOutput").ap()

    with tile.TileContext(nc) as tc, ExitStack() as top:
        dram = top.enter_context(tc.tile_pool(name="dram", bufs=1, space="DRAM"))
        per = top.enter_context(tc.tile_pool(name="per", bufs=1))
        ones_f = per.tile([128, 1], F32)
        nc.gpsimd.memset(ones_f[:], 1.0)
        ones_b = per.tile([128, 1], BF)
        nc.vector.tensor_copy(ones_b[:], ones_f[:])
        ones8 = per.tile([128, 2, 16], F8)
        nc.gpsimd.memset(ones8[:], 1.0)
        eps1 = per.tile([1, 1], F32)
        nc.gpsimd.memset(eps1[:], EPS)
        eC = per.tile([128, 1], F32)
        nc.gpsimd.memset(eC[:], -ECONST)
        from concourse.masks import make_identity
        ident_f = per.tile([128, 128], F32)
        make_identity(nc, ident_f[:])
        ident_b = per.tile([128, 128], BF)
        nc.vector.tensor_copy(ident_b[:], ident_f[:])

        # ---- persistent SBUF: fp32 residual + attention I/O per head ----
        res_ctx = ExitStack()
        res_pool = res_ctx.enter_context(tc.tile_pool(name="res", bufs=1))
        res1 = [res_pool.tile([128, SSH], F32, name=f"res1_{dt}") for dt in range(DT)]
        xr = res_pool.tile([128, DT, SSH], F32, name="xr")

        qk_ctx = ExitStack()
        qk = qk_ctx.enter_context(tc.tile_pool(name="qk", bufs=1))
        qr = [qk.tile([128, S], BF, name=f"qr{h}") for h in range(HPC)]
        kr = [qk.tile([128, S], BF, name=f"kr{h}") for h in range(HPC)]
        V_sb = qk.tile([128, S // 128, EH], F8, name="V_sb")
        att = [qk.tile([128, S], F8, name=f"att{h}") for h in range(HPC)]
        wo_sb = qk.tile([128, DT, DT, 128], F8, name="wo_sb")   # [p, dt, et', c]

        a2a_in = [dram.tile([NC, 128, SSH], F8, name=f"a2a_in{h}") for h in range(HPC)]
        a2a_out = [dram.tile([NC, 128, SSH], F8, name=f"a2a_out{h}") for h in range(HPC)]

        # x / tables / qkv weights: live through P1+P2
        x_ctx = ExitStack()
        xp = x_ctx.enter_context(tc.tile_pool(name="xp", bufs=1))
        x_sb = xp.tile([128, DT, S], F8, name="x_sb")
        wqkv_sb = xp.tile([128, DT, 6 * 128], F8, name="wqkv_sb")
        cs_c = xp.tile([HD, S], F32, name="cs_c")     # cos * rstd / (SW*SX)
        cs_s = xp.tile([HD, S], F32, name="cs_s")
        rstd_bc = xp.tile([128, S], F32, name="rstd_bc")

        def rope(pool, dst, ps, c0):
            # dst[:, c0:c0+1024] = rotate-half rope on ps; rstd + 1/(SW*SX)
            # folded into cs_* so dst comes out at true scale (bf16)
            mc = pool.tile([128, 1024], F32, tag="mc", bufs=1)
            msw = pool.tile([128, 1024], F32, tag="msw", bufs=1)
            nc.vector.tensor_tensor(out=mc[:], in0=ps[:],
                                    in1=cs_c[:, c0:c0 + 1024], op=OP.mult)
            nc.vector.tensor_tensor(out=msw[0:64, :], in0=ps[64:128, :],
                                    in1=cs_s[0:64, c0:c0 + 1024], op=OP.mult)
            nc.vector.tensor_tensor(out=msw[64:128, :], in0=ps[0:64, :],
                                    in1=cs_s[64:128, c0:c0 + 1024], op=OP.mult)
            nc.vector.tensor_tensor(out=dst[:, c0:c0 + 1024], in0=mc[:],
                                    in1=msw[:], op=OP.add)

        def qkv_oc(pool, pspool, h, kind, tag):
            oc = {"q": 0, "k": 2, "v": 4}[kind] + h
            for half in range(2):
                c0 = half * 1024
                ps = pspool.tile([128, 1024], F32, tag=tag, name=f"{kind}{h}_{half}",
                                 bufs=2)
                for dtp in range(DTP):
                    for i in range(2):
                        nc.tensor.matmul(
                            ps[:, i * 512:(i + 1) * 512],
                            wqkv_sb[:, 2 * dtp:2 * dtp + 2, oc * 128:(oc + 1) * 128],
                            x_sb[:, 2 * dtp:2 * dtp + 2,
                                 c0 + i * 512:c0 + (i + 1) * 512],
                            start=(dtp == 0), stop=(dtp == DTP - 1), perf_mode=DR)
                if kind == "q":
                    rope(pool, qr[h], ps[:], c0)
                elif kind == "k":
                    rope(pool, kr[h], ps[:], c0)
                else:
                    # ps = v_unnorm * SW * SX; vsc = *rstd (bf16, big values ok)
                    vsc = pool.tile([128, 1024], BF, tag="vsc", bufs=2)
                    nc.vector.tensor_tensor(out=vsc[:], in0=ps[:],
                                            in1=rstd_bc[:, c0:c0 + 1024], op=OP.mult)
                    for j in range(8):
                        kt = half * 8 + j
                        tr = pspool.tile([128, 128], BF, tag=tag,
                                         name=f"tr{h}{kt}", bufs=2)
                        nc.tensor.transpose(tr[:], vsc[:, j * 128:(j + 1) * 128],
                                            ident_b[:])
                        # V_sb = v_hat * 16 (fp8): tr * 16/(SW*SX)
                        nc.vector.tensor_scalar(
                            V_sb[:, kt, h * 128:(h + 1) * 128], tr[:],
                            16.0 / (SW * SX), None, OP.mult)

        # ============ P1: x load + rmsnorm1 stats + qkv head0 ============
        p1_ctx = ExitStack()
        s1 = p1_ctx.enter_context(tc.tile_pool(name="s1", bufs=1))
        ps1 = p1_ctx.enter_context(tc.tile_pool(name="ps1", bufs=1, space="PSUM"))

        ss_ps = ps1.tile([1, S], F32, tag="stat", name="ss_ps", bufs=1)
        for dt in range(DT):
            nc.sync.dma_start(x_sb[:, dt, :], x8T[dt * 128:(dt + 1) * 128, :])
            nc.sync.dma_start(wqkv_sb[:, dt, :], wqkvT8[dt * 128:(dt + 1) * 128, :])
            if dt == 2:
                nc.sync.dma_start(cs_c[:], cosT)
                nc.sync.dma_start(cs_s[:], sinTs)
            for i in range(2):
                sq = s1.tile([128, 1024], BF, tag="sq", bufs=2)
                nc.vector.tensor_tensor(out=sq[:],
                                        in0=x_sb[:, dt, i * 1024:(i + 1) * 1024],
                                        in1=x_sb[:, dt, i * 1024:(i + 1) * 1024],
                                        op=OP.mult)
                for j in range(2):
                    c = i * 1024 + j * 512
                    nc.tensor.matmul(ss_ps[:, c:c + 512], ones_b[:],
                                     sq[:, j * 512:(j + 1) * 512],
                                     start=(dt == 0), stop=(dt == DT - 1))
        # sq = (16x)^2 = 256 x^2 -> rstd = 1/sqrt(ss/(256 H) + eps)
        for i in range(4):
            sdc = s1.tile([1, 512], F32, tag="sdc", bufs=2)
            nc.scalar.activation(sdc[:], ss_ps[:, i * 512:(i + 1) * 512], AF.Sqrt,
                                 bias=eps1[:], scale=1.0 / (H * SX * SX))
            rsc = s1.tile([1, 512], F32, tag="rsc", bufs=2)
            nc.vector.reciprocal_approx_fast(out=rsc[:], in_=sdc[:])
            nc.gpsimd.partition_broadcast(rstd_bc[:, i * 512:(i + 1) * 512], rsc[:],
                                          channels=128)
        # fold rstd into the rope tables in place
        nc.vector.tensor_tensor(out=cs_c[:], in0=cs_c[:], in1=rstd_bc[:], op=OP.mult)
        nc.vector.tensor_tensor(out=cs_s[:], in0=cs_s[:], in1=rstd_bc[:], op=OP.mult)

        qkv_oc(s1, ps1, 0, "q", "qk")
        qkv_oc(s1, ps1, 0, "k", "qk")
        qkv_oc(s1, ps1, 0, "v", "qk")
        p1_ctx.close()

        # ============ P2: attention h0, qkv h1, attention h1 ============
        p2_ctx = ExitStack()
        s2 = p2_ctx.enter_context(tc.tile_pool(name="s2", bufs=1))
        ps2 = p2_ctx.enter_context(tc.tile_pool(name="ps2", bufs=1, space="PSUM"))

        def attn_head(h):
            for sc in range(2):          # sq chunks of 1024
                c0 = sc * 1024
                av = ps2.tile([128, 1024], F32, tag="av", name=f"av{h}{sc}", bufs=1)
                sm = ps2.tile([1, 1024], F32, tag="sm", name=f"sm{h}{sc}", bufs=1)
                for ktp in range(DTP):
                    e8 = s2.tile([128, 2, 1024], F8, tag="e", bufs=3)
                    for sub in range(2):
                        kt = 2 * ktp + sub
                        st = ps2.tile([128, 1024], F32, tag="big",
                                      name=f"st{h}{sc}", bufs=2)
                        for i in range(2):
                            nc.tensor.matmul(
                                st[:, i * 512:(i + 1) * 512],
                                kr[h][:, kt * 128:(kt + 1) * 128],
                                qr[h][:, c0 + i * 512:c0 + (i + 1) * 512],
                                start=True, stop=True)
                        nc.scalar.activation(e8[:, sub, :], st[:], AF.Exp,
                                             bias=eC[:])
                    for i in range(2):
                        nc.tensor.matmul(sm[:, i * 512:(i + 1) * 512],
                                         ones8[:, :, 0:1],
                                         e8[:, :, i * 512:(i + 1) * 512],
                                         start=(ktp == 0), stop=(ktp == DTP - 1),
                                         perf_mode=DR)
                        nc.tensor.matmul(av[:, i * 512:(i + 1) * 512],
                                         V_sb[:, 2 * ktp:2 * ktp + 2,
                                              h * 128:(h + 1) * 128],
                                         e8[:, :, i * 512:(i + 1) * 512],
                                         start=(ktp == 0), stop=(ktp == DTP - 1),
                                         perf_mode=DR)
                rs = s2.tile([1, 1024], F32, tag="rs", bufs=1)
                nc.vector.reciprocal_approx_fast(out=rs[:], in_=sm[:])
                bc = s2.tile([128, 1024], F32, tag="bc", bufs=1)
                nc.gpsimd.partition_broadcast(bc[:], rs[:], channels=128)
                # av = sum e * (v_hat*16) -> att = attn_true * 16, fp8
                nc.vector.tensor_tensor(out=att[h][:, c0:c0 + 1024], in0=av[:],
                                        in1=bc[:], op=OP.mult)
            for j in range(NC):
                nc.sync.dma_start(a2a_in[h][j], att[h][:, j * SSH:(j + 1) * SSH])
            nc.gpsimd.collective_compute("AllToAll", OP.bypass,
                                         ins=[a2a_in[h][:]], outs=[a2a_out[h][:]],
                                         replica_groups=RG)

        attn_head(0)
        # prefetch full wo (fp8, 4MB) during attention h1 epoch
        for dt in range(DT):
            nc.sync.dma_start(wo_sb[:, dt], woTt8[dt])
        qkv_oc(s2, ps2, 1, "q", "big")
        qkv_oc(s2, ps2, 1, "k", "big")
        qkv_oc(s2, ps2, 1, "v", "big")
        for dt in range(DT):   # residual slice; needed from o_proj onwards
            nc.sync.dma_start(xr[:, dt, :], xTrs[dt * 128:(dt + 1) * 128, :])
        attn_head(1)
        p2_ctx.close()
        x_ctx.close()

        # ============ P3: o_proj + residual + rmsnorm2 + AG ============
        h2_ctx = ExitStack()
        h2p = h2_ctx.enter_context(tc.tile_pool(name="h2p", bufs=1))
        h2 = [h2p.tile([128, SSH], BF, name=f"h2_{dt}") for dt in range(DT)]
        h2f8 = [h2p.tile([128, SSH], F8, name=f"h2f8_{dt}") for dt in range(DT)]
        ag8_in = [dram.tile([128, DT, 128], F8, name=f"ag8_in{t}") for t in range(2)]
        ag8_out = [dram.tile([NC, 128, DT, 128], F8, addr_space="Shared",
                             name=f"ag8_out{t}") for t in range(2)]
        ag_in = [dram.tile([128, DT, 128], BF, name=f"ag_in{t}") for t in range(2)]
        ag_out = [dram.tile([NC, 128, DT, 128], BF, addr_space="Shared",
                            name=f"ag_out{t}") for t in range(2)]
        with tc.tile_pool(name="s3", bufs=1) as s3, \
             tc.tile_pool(name="ps3", bufs=1, space="PSUM") as ps3:
            # attg slot et' = h*8 + j  (head-major; matches woTt8 permute)
            attg = s3.tile([128, DT, SSH], F8, tag="attg")
            for h in range(HPC):
                for j in range(NC):
                    nc.sync.dma_start(attg[:, h * 8 + j, :], a2a_out[h][j])
            ss2_ps = ps3.tile([1, SSH], F32, tag="ss2", name="ss2_ps")
            for dtq in range(4):
                o_ps = ps3.tile([128, 1024], F32, tag="o", name=f"o_ps{dtq}",
                                bufs=2)
                for dtl in range(4):
                    dt = 4 * dtq + dtl
                    for ep in range(DTP):
                        nc.tensor.matmul(
                            o_ps[:, dtl * 256:(dtl + 1) * 256],
                            wo_sb[:, dt, 2 * ep:2 * ep + 2, :],
                            attg[:, 2 * ep:2 * ep + 2, :],
                            start=(ep == 0), stop=(ep == DTP - 1), perf_mode=DR)
                for dtl in range(4):
                    dt = 4 * dtq + dtl
                    obf = s3.tile([128, SSH], F32, tag="obf", bufs=2)
                    # o_ps = o_true * SW * 16
                    nc.vector.tensor_scalar(obf[:],
                                            o_ps[:, dtl * 256:(dtl + 1) * 256],
                                            1.0 / (SW * 16.0), None, OP.mult)
                    nc.vector.tensor_tensor(out=res1[dt][:], in0=obf[:],
                                            in1=xr[:, dt, :], op=OP.add)
                    sq2 = s3.tile([128, SSH], BF, tag="sq2", bufs=2)
                    nc.vector.tensor_tensor(out=sq2[:], in0=res1[dt][:],
                                            in1=res1[dt][:], op=OP.mult)
                    nc.tensor.matmul(ss2_ps[:], ones_b[:], sq2[:],
                                     start=(dt == 0), stop=(dt == DT - 1))
            sd2 = s3.tile([1, SSH], F32, name="sd2")
            nc.scalar.activation(sd2[:], ss2_ps[:], AF.Sqrt, bias=eps1[:],
                                 scale=1.0 / H)
            rstd2 = s3.tile([1, SSH], F32, name="rstd2")
            nc.vector.reciprocal_approx_fast(out=rstd2[:], in_=sd2[:])
            rstd2_bc = s3.tile([128, SSH], F32, name="rstd2_bc")
            nc.gpsimd.partition_broadcast(rstd2_bc[:], rstd2[:], channels=128)
            for dt in range(DT):
                nc.vector.tensor_tensor(out=h2[dt][:], in0=res1[dt][:],
                                        in1=rstd2_bc[:], op=OP.mult)
                nc.vector.tensor_scalar(h2f8[dt][:], h2[dt][:], SX, None, OP.mult)
            # AG per token-half; fp8 first so gate can start earliest
            for ts in range(2):
                for dt in range(DT):
                    nc.sync.dma_start(ag8_in[ts][:, dt, :],
                                      h2f8[dt][:, ts * 128:(ts + 1) * 128])
                nc.gpsimd.collective_compute("AllGather", OP.bypass,
                                             ins=[ag8_in[ts][:]],
                                             outs=[ag8_out[ts][:]],
                                             replica_groups=RG)
                for dt in range(DT):
                    nc.sync.dma_start(ag_in[ts][:, dt, :],
                                      h2[dt][:, ts * 128:(ts + 1) * 128])
                nc.gpsimd.collective_compute("AllGather", OP.bypass,
                                             ins=[ag_in[ts][:]], outs=[ag_out[ts][:]],
                                             replica_groups=RG)
        h2_ctx.close()
        qk_ctx.close()

        # ============ P4: MLP per token-half, RS chunked (last one small) ====
        CHUNKS = {0: [8, 8], 1: [8, 6, 2]}
        rs_in = {(t, g): dram.tile([NC, 128, n, 128], BF, name=f"rs_in{t}{g}")
                 for t in range(2) for g, n in enumerate(CHUNKS[t])}
        rs_out = {(t, g): dram.tile([128, n, 128], BF, name=f"rs_out{t}{g}")
                  for t in range(2) for g, n in enumerate(CHUNKS[t])}
        with tc.tile_pool(name="s4", bufs=1) as s4, \
             tc.tile_pool(name="ps4", bufs=1, space="PSUM") as ps4:
            for ts in range(2):
                # token order within the half is src-core-major (j-major)
                h2g8 = s4.tile([128, DT, NC * 128], F8, tag="h2g8", bufs=2,
                               name=f"h2g8{ts}")
                h2g = s4.tile([128, DT, NC * 128], BF, tag="h2g", bufs=1,
                              name=f"h2g{ts}")
                for j in range(NC):
                    nc.sync.dma_start(h2g8[:, :, j * 128:(j + 1) * 128],
                                      ag8_out[ts][j])
                    nc.sync.dma_start(h2g[:, :, j * 128:(j + 1) * 128],
                                      ag_out[ts][j])
                act_t = s4.tile([128, MT, 1024], BF, tag="act", bufs=1,
                                name=f"act{ts}")
                for mt in range(MT):
                    wg_t = s4.tile([128, DT, 128], F8, tag="wg", bufs=2)
                    wu_t = s4.tile([128, DT, 128], BF, tag="wu", bufs=2)
                    nc.sync.dma_start(wg_t[:], wgTt8[mt])
                    nc.sync.dma_start(wu_t[:], wuTt[mt])
                    g_ps = ps4.tile([128, 1024], F32, tag="g", name=f"g{ts}{mt}",
                                    bufs=2)
                    u_ps = ps4.tile([128, 1024], F32, tag="u", name=f"u{ts}{mt}",
                                    bufs=2)
                    for dtp in range(DTP):
                        for i in range(2):
                            nc.tensor.matmul(
                                g_ps[:, i * 512:(i + 1) * 512],
                                wg_t[:, 2 * dtp:2 * dtp + 2, :],
                                h2g8[:, 2 * dtp:2 * dtp + 2,
                                     i * 512:(i + 1) * 512],
                                start=(dtp == 0), stop=(dtp == DTP - 1),
                                perf_mode=DR)
                    for dt in range(DT):
                        for i in range(2):
                            nc.tensor.matmul(
                                u_ps[:, i * 512:(i + 1) * 512], wu_t[:, dt, :],
                                h2g[:, dt, i * 512:(i + 1) * 512],
                                start=(dt == 0), stop=(dt == DT - 1))
                    gs = s4.tile([128, 1024], BF, tag="gs", bufs=2)
                    # g_ps = g_true * SW * SX
                    nc.scalar.activation(gs[:], g_ps[:], AF.Sigmoid,
                                         scale=1.0 / (SW * SX))
                    nc.vector.tensor_tensor(out=act_t[:, mt, :], in0=u_ps[:],
                                            in1=gs[:], op=OP.mult)
                # down: contraction over mt; RS fired per chunk
                dt0 = 0
                for grp, nch in enumerate(CHUNKS[ts]):
                    dn = s4.tile([128, 8, 1024], BF, tag="dn", bufs=2,
                                 name=f"dn{ts}{grp}")
                    for k in range(nch):
                        dt = dt0 + k
                        wd_t = s4.tile([128, MT, 128], BF, tag="wd", bufs=3)
                        nc.sync.dma_start(wd_t[:], wdTt[dt])
                        d_ps = ps4.tile([128, 1024], F32,
                                        tag=("g" if dt % 2 == 0 else "u"),
                                        name=f"d{ts}{dt}", bufs=2)
                        for mt in range(MT):
                            for i in range(2):
                                nc.tensor.matmul(
                                    d_ps[:, i * 512:(i + 1) * 512], wd_t[:, mt, :],
                                    act_t[:, mt, i * 512:(i + 1) * 512],
                                    start=(mt == 0), stop=(mt == MT - 1))
                        nc.vector.tensor_copy(dn[:, k, :], d_ps[:])
                    for j in range(NC):
                        nc.sync.dma_start(
                            rs_in[ts, grp][j],
                            dn[:, 0:nch, j * 128:(j + 1) * 128])
                    nc.gpsimd.collective_compute(
                        "ReduceScatter", OP.add, ins=[rs_in[ts, grp][:]],
                        outs=[rs_out[ts, grp][:]], replica_groups=RG)
                    dt0 += nch
                # finalize each chunk as its RS lands (overlaps next work)
                dt0 = 0
                for grp, nch in enumerate(CHUNKS[ts]):
                    rsb = s4.tile([128, 8, 128], BF, tag="rsb", bufs=2)
                    nc.sync.dma_start(rsb[:, 0:nch, :], rs_out[ts, grp][:])
                    for k in range(nch):
                        dt = dt0 + k
                        fin = s4.tile([128, 128], F32, tag="fin", bufs=4)
                        nc.vector.tensor_tensor(
                            out=fin[:], in0=rsb[:, k, :],
                            in1=res1[dt][:, ts * 128:(ts + 1) * 128], op=OP.add)
                        nc.sync.dma_start(out_sh[dt * 128:(dt + 1) * 128,
                                                 ts * 128:(ts + 1) * 128], fin[:])
                    dt0 += nch

        res_ctx.close()

    nc.compile()
    return nc


_PROG = None


def _get_program():
    global _PROG
    if _PROG is None:
        _PROG = _build_program()
    return _PROG


def _prep_inputs(x, norm1_w, wq, wk, wv, wo, norm2_w, w_gate, w_up, w_down, cos, sin):
    x = np.asarray(x, dtype=np.float32)
    xT = np.ascontiguousarray(x.reshape(S, H).T)                       # [H, S]
    x8T = _q8(xT, SX)
    cosT = np.ascontiguousarray(np.asarray(cos, np.float32).T) / (SW * SX)
    sinT = np.ascontiguousarray(np.asarray(sin, np.float32).T) / (SW * SX)
    sinTs = sinT.copy()
    sinTs[0:HD // 2] = -sinTs[0:HD // 2]       # rotate_half sign for lo rows
    n1 = np.asarray(norm1_w, np.float32)
    n2 = np.asarray(norm2_w, np.float32)
    wq = np.asarray(wq, np.float32) * n1[None, :] / np.sqrt(np.float32(HD))
    wk = np.asarray(wk, np.float32) * n1[None, :]
    wv = np.asarray(wv, np.float32) * n1[None, :]
    wg = np.asarray(w_gate, np.float32) * n2[None, :]
    wu = np.asarray(w_up, np.float32) * n2[None, :]
    wo = np.asarray(wo, np.float32)
    wd = np.asarray(w_down, np.float32)

    woT8 = _q8(wo.T, SW)                                               # [e=H, d=H]
    # e-tile permute to head-major: slot et' = h*8 + j holds e-tile 2j+h
    woTt = woT8.reshape(DT, 128, DT, 128)
    perm = [2 * (e % 8) + (e // 8) for e in range(DT)]
    woTt8 = np.ascontiguousarray(
        woTt[perm].transpose(2, 1, 0, 3))                              # [dt, p, et', c]

    in_maps = []
    for c in range(NC):
        e0 = c * EH
        m0 = c * MSH
        wqkv = np.concatenate([wq[e0:e0 + EH, :], wk[e0:e0 + EH, :],
                               wv[e0:e0 + EH, :]], axis=0)             # [768, H]
        wqkvT8 = _q8(np.ascontiguousarray(wqkv.T), SW)                 # [H, 768]
        wgT8 = _q8(wg[m0:m0 + MSH, :].T, SW)                           # [H, MSH]
        wuT = wu[m0:m0 + MSH, :].T.astype(BF_NP)
        wdT = wd[:, m0:m0 + MSH].T.astype(BF_NP)                       # [MSH, H]
        in_maps.append({
            "x8T": x8T,
            "xTrs": np.ascontiguousarray(xT[:, c * SSH:(c + 1) * SSH]),
            "cosT": cosT, "sinTs": sinTs,
            "wqkvT8": wqkvT8,
            "woTt8": woTt8,
            "wgTt8": np.ascontiguousarray(
                wgT8.reshape(DT, 128, MT, 128).transpose(2, 1, 0, 3)),  # [mt,p,dt,c]
            "wuTt": np.ascontiguousarray(
                wuT.reshape(DT, 128, MT, 128).transpose(2, 1, 0, 3)),
            "wdTt": np.ascontiguousarray(
                wdT.reshape(MT, 128, DT, 128).transpose(2, 1, 0, 3)),  # [dt,p,mt,c]
        })
    return in_maps


def kernel(x, norm1_w, wq, wk, wv, wo, norm2_w, w_gate, w_up, w_down, cos, sin,
           _want_results=False):
    in_maps = _prep_inputs(x, norm1_w, wq, wk, wv, wo, norm2_w,
                           w_gate, w_up, w_down, cos, sin)
    prog = _get_program()
    res = run_bass_kernel_spmd(prog, in_maps, list(range(NC)))
    out = np.empty((B, S, H), dtype=np.float32)
    for c in range(NC):
        out[0, c * SSH:(c + 1) * SSH, :] = res.results[c]["out_sh"].T
    if _want_results:
        return out, res
    return out


# revision 10
# speedup vs baseline: 1.1512x; 1.0159x over previous
"""Trainium2 Bass kernel for nn_DeepseekLayer (dense transformer layer).

Sharding (8 cores): Megatron-style TP, fp8(e4m3)+bf16 datapath, fp32 PSUM.
  - qkv head-sharded (2 heads/core) over full S; x resident in SBUF as fp8
    (x*16); qkv/o/gate weights fp8 (*2^10), matmuls in DoubleRow perf mode
    (2 k-subtiles of 128 per instruction, ~1.5x bf16 throughput).
  - rmsnorm1 folded into rope tables / V-scale; rstd computed on device;
    x^2 computed on the (otherwise idle) ACT engine.
  - attention: transposed-softmax layout (scores [sk, sq]) in bf16 on
    512-wide sq chunks, exp(score-4) written as fp8 by ACT, softmax sums +
    probs@V as fp8 DoubleRow matmuls, fast-reciprocal normalize, per-head
    AllToAll (fp8, att*16).  qkv for head 1 is interleaved into head 0's
    attention instruction stream to keep PE and ACT both fed.
  - o_proj fp8 DoubleRow split by head: the head-0 half runs during the
    AllToAll(h1) flight, the head-1 half right after it lands.
  - rmsnorm2 + AllGather pipelined per token-half (fp8 h2*16 for gate,
    bf16 for up) with contiguous staging DMAs; MLP(ts0) overlaps the
    ts1 bridge.  gate fp8 DoubleRow, up/down bf16 (fp8 there would break
    the 2e-2 error budget) -> chunked bf16 ReduceScatter (j-major dn
    layout for contiguous staging, small last chunk) -> residual add.
"""
import numpy as np
from contextlib import ExitStack

import ml_dtypes
from concourse import bacc
import concourse.tile as tile
import concourse.mybir as mybir
from concourse.bass_utils import run_bass_kernel_spmd

F32 = mybir.dt.float32
BF = mybir.dt.bfloat16
F8 = mybir.dt.float8e4
AF = mybir.ActivationFunctionType
OP = mybir.AluOpType
DR = mybir.MatmulPerfMode.DoubleRow

H = 2048          # hidden
NH = 16           # heads
HD = 128          # head dim
MLP = 8192
S = 2048          # sequence
B = 1
EPS = 1e-6
NC = 8            # cores
HPC = NH // NC    # heads per core = 2
EH = HPC * HD     # qkv out dims per core = 256
MSH = MLP // NC   # mlp dims per core = 1024
SSH = S // NC     # tokens per shard = 256
RG = [list(range(NC))]
DT = H // 128     # 16 d-tiles
DTP = DT // 2     # 8 d-tile pairs
MT = MSH // 128   # 8 m-tiles per core
BF_NP = ml_dtypes.bfloat16
F8_NP = ml_dtypes.float8_e4m3   # TRN fp8e4: max +-240

SX = 16.0         # x / activation fp8 scale
SW = 1024.0       # weight fp8 scale (2^10)
ECONST = 4.0      # exp(score - ECONST); cancels in softmax


def _q8(a, scale):
    return np.clip(np.asarray(a, np.float32) * scale, -240.0, 240.0).astype(F8_NP)


def _merge(a_thunks, b_thunks):
    """Emit a-thunks in order with b-thunks proportionally interspersed."""
    na, nb = len(a_thunks), len(b_thunks)
    ib = 0
    for i, a in enumerate(a_thunks):
        a()
        while ib < nb and ib * na < (i + 1) * nb:
            b_thunks[ib]()
            ib += 1
    while ib < nb:
        b_thunks[ib]()
        ib += 1


def _build_program():
    nc = bacc.Bacc(trn_type="TRN2", target_bir_lowering=False, debug=False,
                   num_devices=NC)

    def inp(name, shape, dt):
        return nc.dram_tensor(name, shape, dt, kind="ExternalInput").ap()

    x8T = inp("x8T", [H, S], F8)                # x.T * 16, fp8
    xTrs = inp("xTrs", [H, SSH], F32)           # this core's token-shard, f32
    cosT = inp("cosT", [HD, S], F32)            # cos.T / (SW*SX)
    sinTs = inp("sinTs", [HD, S], F32)          # sin.T / (SW*SX), rows 0:63 neg
    wqkvT8 = inp("wqkvT8", [H, 6 * 128], F8)    # cols: q0,q1,k0,k1,v0,v1 (*SW)
    woTt8 = inp("woTt8", [DT, 128, DT, 128], F8)  # wo.T [dt, p, et', c] head-major
    wgTt8 = inp("wgTt8", [MT, 128, DT, 128], F8)  # (wg*n2).T shard *SW
    wuTt = inp("wuTt", [MT, 128, DT, 128], BF)
    wdTt = inp("wdTt", [DT, 128, MT, 128], BF)
    out_sh = nc.dram_tensor("out_sh", [H, SSH], F32, kind="ExternalOutput").ap()

    with tile.TileContext(nc) as tc, ExitStack() as top:
        dram = top.enter_context(tc.tile_pool(name="dram", bufs=1, space="DRAM"))
        per = top.enter_context(tc.tile_pool(name="per", bufs=1))
        ones_f = per.tile([128, 1], F32)
        nc.gpsimd.memset(ones_f[:], 1.0)
        ones_b = per.tile([128, 1], BF)
        nc.vector.tensor_copy(ones_b[:], ones_f[:])
        ones8 = per.tile([128, 2, 16], F8)
        nc.gpsimd.memset(ones8[:], 1.0)
        eps1 = per.tile([1, 1], F32)
        nc.gpsimd.memset(eps1[:], EPS)
        eC = per.tile([128, 1], F32)
        nc.gpsimd.memset(eC[:], -ECONST)
        from concourse.masks import make_identity
        ident_f = per.tile([128, 128], F32)
        make_identity(nc, ident_f[:])
        ident_b = per.tile([128, 128], BF)
        nc.vector.tensor_copy(ident_b[:], ident_f[:])

        # ---- persistent SBUF: fp32 residual + o accumulator ----
        res_ctx = ExitStack()
        res_pool = res_ctx.enter_context(tc.tile_pool(name="res", bufs=1))
        res1 = [res_pool.tile([128, SSH], F32, name=f"res1_{dt}") for dt in range(DT)]

        qk_ctx = ExitStack()
        qk = qk_ctx.enter_context(tc.tile_pool(name="qk", bufs=1))
        qr = [qk.tile([128, S], BF, name=f"qr{h}") for h in range(HPC)]
        kr = [qk.tile([128, S], BF, name=f"kr{h}") for h in range(HPC)]
        V_sb = qk.tile([128, S // 128, EH], F8, name="V_sb")
        att = [qk.tile([128, S], F8, name=f"att{h}") for h in range(HPC)]
        wo_sb = qk.tile([128, DT, DT, 128], F8, name="wo_sb")   # [p, dt, et', c]
        attg = [qk.tile([128, DTP, SSH], F8, name=f"attg{h}")
                for h in range(HPC)]                            # slot j of head h

        a2a_in = [dram.tile([NC, 128, SSH], F8, name=f"a2a_in{h}") for h in range(HPC)]
        a2a_out = [dram.tile([NC, 128, SSH], F8, name=f"a2a_out{h}") for h in range(HPC)]

        # x / tables / qkv weights: live through P1+P2
        x_ctx = ExitStack()
        xp = x_ctx.enter_context(tc.tile_pool(name="xp", bufs=1))
        x_sb = xp.tile([128, DT, S], F8, name="x_sb")
        wqkv_sb = xp.tile([128, DT, 6 * 128], F8, name="wqkv_sb")
        cs_c = xp.tile([HD, S], F32, name="cs_c")     # cos * rstd / (SW*SX)
        cs_s = xp.tile([HD, S], F32, name="cs_s")
        rstd_bc = xp.tile([128, S], F32, name="rstd_bc")
        xr = xp.tile([128, DT, SSH], F32, name="xr")
        o_acc = xp.tile([128, DT, SSH], F32, name="o_acc")

        def rope(pool, dst, ps, c0):
            mc = pool.tile([128, 1024], F32, tag="mc", bufs=1)
            msw = pool.tile([128, 1024], F32, tag="msw", bufs=1)
            nc.vector.tensor_tensor(out=mc[:], in0=ps[:],
                                    in1=cs_c[:, c0:c0 + 1024], op=OP.mult)
            nc.vector.tensor_tensor(out=msw[0:64, :], in0=ps[64:128, :],
                                    in1=cs_s[0:64, c0:c0 + 1024], op=OP.mult)
            nc.vector.tensor_tensor(out=msw[64:128, :], in0=ps[0:64, :],
                                    in1=cs_s[64:128, c0:c0 + 1024], op=OP.mult)
            nc.vector.tensor_tensor(out=dst[:, c0:c0 + 1024], in0=mc[:],
                                    in1=msw[:], op=OP.add)

        def qkv_units(pool, pspool, h, tag):
            """Thunk list: qkv projections for head h, 4-MM units."""
            units = []
            for kind in ("q", "k", "v"):
                oc = {"q": 0, "k": 2, "v": 4}[kind] + h
                for half in range(2):
                    c0 = half * 1024
                    ps = pspool.tile([128, 1024], F32, tag=tag,
                                     name=f"{kind}{h}_{half}", bufs=2)

                    def mm(dtp0, ps=ps, oc=oc, c0=c0):
                        for dtp in range(dtp0, dtp0 + 2):
                            for i in range(2):
                                nc.tensor.matmul(
                                    ps[:, i * 512:(i + 1) * 512],
                                    wqkv_sb[:, 2 * dtp:2 * dtp + 2,
                                            oc * 128:(oc + 1) * 128],
                                    x_sb[:, 2 * dtp:2 * dtp + 2,
                                         c0 + i * 512:c0 + (i + 1) * 512],
                                    start=(dtp == 0), stop=(dtp == DTP - 1),
                                    perf_mode=DR)
                    for dtp0 in range(0, DTP, 2):
                        units.append(lambda dtp0=dtp0, mm=mm: mm(dtp0))

                    def fin(ps=ps, kind=kind, c0=c0, half=half, h=h):
                        if kind == "q":
                            rope(pool, qr[h], ps[:], c0)
                        elif kind == "k":
                            rope(pool, kr[h], ps[:], c0)
                        else:
                            vsc = pool.tile([128, 1024], BF, tag="vsc", bufs=2)
                            nc.vector.tensor_tensor(out=vsc[:], in0=ps[:],
                                                    in1=rstd_bc[:, c0:c0 + 1024],
                                                    op=OP.mult)
                            for j in range(8):
                                kt = half * 8 + j
                                tr = pspool.tile([128, 128], BF, tag=tag,
                                                 name=f"tr{h}{kt}", bufs=2)
                                nc.tensor.transpose(tr[:],
                                                    vsc[:, j * 128:(j + 1) * 128],
                                                    ident_b[:])
                                nc.vector.tensor_scalar(
                                    V_sb[:, kt, h * 128:(h + 1) * 128], tr[:],
                                    16.0 / (SW * SX), None, OP.mult)
                    units.append(fin)
            return units

        # ============ P1: x load + rmsnorm1 stats + qkv head0 ============
        p1_ctx = ExitStack()
        s1 = p1_ctx.enter_context(tc.tile_pool(name="s1", bufs=1))
        ps1 = p1_ctx.enter_context(tc.tile_pool(name="ps1", bufs=1, space="PSUM"))

        ss_ps = ps1.tile([1, S], F32, tag="stat", name="ss_ps", bufs=1)
        for dt in range(DT):
            nc.sync.dma_start(x_sb[:, dt, :], x8T[dt * 128:(dt + 1) * 128, :])
            nc.sync.dma_start(wqkv_sb[:, dt, :], wqkvT8[dt * 128:(dt + 1) * 128, :])
            if dt == 2:
                nc.scalar.dma_start(cs_c[:], cosT)
                nc.scalar.dma_start(cs_s[:], sinTs)
            for i in range(2):
                sq = s1.tile([128, 1024], BF, tag="sq", bufs=2)
                nc.scalar.activation(sq[:], x_sb[:, dt, i * 1024:(i + 1) * 1024],
                                     AF.Square)
                for j in range(2):
                    c = i * 1024 + j * 512
                    nc.tensor.matmul(ss_ps[:, c:c + 512], ones_b[:],
                                     sq[:, j * 512:(j + 1) * 512],
                                     start=(dt == 0), stop=(dt == DT - 1))
        # sq = (16x)^2 -> rstd = 1/sqrt(ss/(256 H) + eps)
        for i in range(4):
            sdc = s1.tile([1, 512], F32, tag="sdc", bufs=2)
            nc.scalar.activation(sdc[:], ss_ps[:, i * 512:(i + 1) * 512], AF.Sqrt,
                                 bias=eps1[:], scale=1.0 / (H * SX * SX))
            rsc = s1.tile([1, 512], F32, tag="rsc", bufs=2)
            nc.vector.reciprocal_approx_fast(out=rsc[:], in_=sdc[:])
            nc.gpsimd.partition_broadcast(rstd_bc[:, i * 512:(i + 1) * 512], rsc[:],
                                          channels=128)
        nc.vector.tensor_tensor(out=cs_c[:], in0=cs_c[:], in1=rstd_bc[:], op=OP.mult)
        nc.vector.tensor_tensor(out=cs_s[:], in0=cs_s[:], in1=rstd_bc[:], op=OP.mult)

        for u in qkv_units(s1, ps1, 0, "qk"):
            u()
        p1_ctx.close()

        # ============ P2: attn h0 (+qkv h1 interleaved), o split, a2a ============
        p2_ctx = ExitStack()
        s2 = p2_ctx.enter_context(tc.tile_pool(name="s2", bufs=1))
        ps2 = p2_ctx.enter_context(tc.tile_pool(name="ps2", bufs=1, space="PSUM"))

        def attn_units(h):
            """Thunk list for attention head h on 512-wide sq chunks."""
            units = []
            for sc in range(4):
                c0 = sc * 512
                box = {}

                def open_chunk(box=box, h=h, sc=sc):
                    box["av"] = ps2.tile([128, 512], F32, tag="av",
                                         name=f"av{h}{sc}", bufs=1)
                    box["sm"] = ps2.tile([1, 512], F32, tag="sm",
                                         name=f"sm{h}{sc}", bufs=1)
                units.append(open_chunk)

                def unit(ktp, box=box, h=h, c0=c0):
                    e8 = s2.tile([128, 2, 512], F8, tag="e", bufs=3)
                    for sub in range(2):
                        kt = 2 * ktp + sub
                        st = ps2.tile([128, 512], F32, tag="st",
                                      name=f"st{h}", bufs=2)
                        nc.tensor.matmul(st[:],
                                         kr[h][:, kt * 128:(kt + 1) * 128],
                                         qr[h][:, c0:c0 + 512],
                                         start=True, stop=True)
                        nc.scalar.activation(e8[:, sub, :], st[:], AF.Exp,
                                             bias=eC[:])
                    nc.tensor.matmul(box["sm"][:], ones8[:, :, 0:1], e8[:],
                                     start=(ktp == 0), stop=(ktp == DTP - 1),
                                     perf_mode=DR)
                    nc.tensor.matmul(box["av"][:],
                                     V_sb[:, 2 * ktp:2 * ktp + 2,
                                          h * 128:(h + 1) * 128],
                                     e8[:],
                                     start=(ktp == 0), stop=(ktp == DTP - 1),
                                     perf_mode=DR)
                for ktp in range(DTP):
                    units.append(lambda ktp=ktp, unit=unit: unit(ktp))

                def close_chunk(box=box, h=h, c0=c0):
                    rs = s2.tile([1, 512], F32, tag="rs", bufs=2)
                    nc.vector.reciprocal_approx_fast(out=rs[:], in_=box["sm"][:])
                    bc = s2.tile([128, 512], F32, tag="bc", bufs=2)
                    nc.gpsimd.partition_broadcast(bc[:], rs[:], channels=128)
                    nc.vector.tensor_tensor(out=att[h][:, c0:c0 + 512],
                                            in0=box["av"][:], in1=bc[:], op=OP.mult)
                units.append(close_chunk)
            return units

        def fire_a2a(h):
            for j in range(NC):
                nc.sync.dma_start(a2a_in[h][j], att[h][:, j * SSH:(j + 1) * SSH])
            nc.gpsimd.collective_compute("AllToAll", OP.bypass,
                                         ins=[a2a_in[h][:]], outs=[a2a_out[h][:]],
                                         replica_groups=RG)

        # prefetch wo (4MB fp8) + residual: no deps, DMA ring runs them early
        for dt in range(DT):
            nc.sync.dma_start(wo_sb[:, dt], woTt8[dt])
            nc.sync.dma_start(xr[:, dt, :], xTrs[dt * 128:(dt + 1) * 128, :])
        # attn h0 with qkv h1 interleaved into its instruction stream
        _merge(attn_units(0), qkv_units(s2, ps2, 1, "qk2"))
        fire_a2a(0)
        for j in range(NC):
            nc.sync.dma_start(attg[0][:, j, :], a2a_out[0][j])

        for u in attn_units(1):
            u()

        # o_proj head-0 half: runs while AllToAll(h1) is in flight.
        # o_ps = o_true * SW * 16; o_acc = o_h0 / (SW*16) + xr
        for dtq in range(DTP):
            o_ps = ps2.tile([128, 512], F32, tag="qk2", name=f"o0_{dtq}", bufs=2)
            for dtl in range(2):
                dt = 2 * dtq + dtl
                for ep in range(4):
                    nc.tensor.matmul(
                        o_ps[:, dtl * 256:(dtl + 1) * 256],
                        wo_sb[:, dt, 2 * ep:2 * ep + 2, :],
                        attg[0][:, 2 * ep:2 * ep + 2, :],
                        start=(ep == 0), stop=(ep == 3), perf_mode=DR)
            for dtl in range(2):
                dt = 2 * dtq + dtl
                nc.vector.tensor_scalar(o_acc[:, dt, :],
                                        o_ps[:, dtl * 256:(dtl + 1) * 256],
                                        1.0 / (SW * 16.0), None, OP.mult)
                nc.vector.tensor_tensor(out=o_acc[:, dt, :], in0=o_acc[:, dt, :],
                                        in1=xr[:, dt, :], op=OP.add)
        fire_a2a(1)
        for j in range(NC):
            nc.sync.dma_start(attg[1][:, j, :], a2a_out[1][j])
        # o_proj head-1 half (waits on AllToAll(h1)) + residual
        for dtq in range(DTP):
            o_ps = ps2.tile([128, 512], F32, tag="qk2", name=f"o1_{dtq}", bufs=2)
            for dtl in range(2):
                dt = 2 * dtq + dtl
                for ep in range(4):
                    nc.tensor.matmul(
                        o_ps[:, dtl * 256:(dtl + 1) * 256],
                        wo_sb[:, dt, 2 * (ep + 4):2 * (ep + 4) + 2, :],
                        attg[1][:, 2 * ep:2 * ep + 2, :],
                        start=(ep == 0), stop=(ep == 3), perf_mode=DR)
            for dtl in range(2):
                dt = 2 * dtq + dtl
                obf = s2.tile([128, SSH], F32, tag="obf", bufs=2)
                nc.vector.tensor_scalar(obf[:], o_ps[:, dtl * 256:(dtl + 1) * 256],
                                        1.0 / (SW * 16.0), None, OP.mult)
                nc.vector.tensor_tensor(out=res1[dt][:], in0=obf[:],
                                        in1=o_acc[:, dt, :], op=OP.add)
        p2_ctx.close()
        x_ctx.close()

        # ============ P3: rmsnorm2 + AG, pipelined per token-half ============
        ag8_in = [dram.tile([128, DT, 128], F8, name=f"ag8_in{t}") for t in range(2)]
        ag8_out = [dram.tile([NC, 128, DT, 128], F8, addr_space="Shared",
                             name=f"ag8_out{t}") for t in range(2)]
        ag_in = [dram.tile([128, DT, 128], BF, name=f"ag_in{t}") for t in range(2)]
        ag_out = [dram.tile([NC, 128, DT, 128], BF, addr_space="Shared",
                            name=f"ag_out{t}") for t in range(2)]
        with tc.tile_pool(name="s3", bufs=1) as s3, \
             tc.tile_pool(name="ps3", bufs=1, space="PSUM") as ps3:
            for ts in range(2):
                tsl = slice(ts * 128, (ts + 1) * 128)
                ss2_ps = ps3.tile([1, 128], F32, tag="ss2", name=f"ss2_{ts}",
                                  bufs=2)
                for dt in range(DT):
                    sq2 = s3.tile([128, 128], BF, tag="sq2", bufs=3)
                    nc.vector.tensor_tensor(out=sq2[:], in0=res1[dt][:, tsl],
                                            in1=res1[dt][:, tsl], op=OP.mult)
                    nc.tensor.matmul(ss2_ps[:], ones_b[:], sq2[:],
                                     start=(dt == 0), stop=(dt == DT - 1))
                sd2 = s3.tile([1, 128], F32, tag="sd2", bufs=2)
                nc.scalar.activation(sd2[:], ss2_ps[:], AF.Sqrt, bias=eps1[:],
                                     scale=1.0 / H)
                rstd2 = s3.tile([1, 128], F32, tag="rstd2", bufs=2)
                nc.vector.reciprocal_approx_fast(out=rstd2[:], in_=sd2[:])
                rstd2_bc = s3.tile([128, 128], F32, tag="r2bc", bufs=2)
                nc.gpsimd.partition_broadcast(rstd2_bc[:], rstd2[:], channels=128)
                h2t = s3.tile([128, DT, 128], BF, tag="h2t", bufs=2,
                              name=f"h2t{ts}")
                h8t = s3.tile([128, DT, 128], F8, tag="h8t", bufs=2,
                              name=f"h8t{ts}")
                for dt in range(DT):
                    nc.vector.tensor_tensor(out=h2t[:, dt, :], in0=res1[dt][:, tsl],
                                            in1=rstd2_bc[:], op=OP.mult)
                    nc.vector.tensor_scalar(h8t[:, dt, :], h2t[:, dt, :], SX,
                                            None, OP.mult)
                # contiguous staging DMAs + AGs; fp8 first (gate needs it first)
                nc.sync.dma_start(ag8_in[ts][:], h8t[:])
                nc.gpsimd.collective_compute("AllGather", OP.bypass,
                                             ins=[ag8_in[ts][:]],
                                             outs=[ag8_out[ts][:]],
                                             replica_groups=RG)
                nc.sync.dma_start(ag_in[ts][:], h2t[:])
                nc.gpsimd.collective_compute("AllGather", OP.bypass,
                                             ins=[ag_in[ts][:]], outs=[ag_out[ts][:]],
                                             replica_groups=RG)
        qk_ctx.close()

        # ============ P4: MLP per token-half, RS chunked (last one small) ====
        CHUNKS = {0: [8, 8], 1: [8, 6, 2]}
        rs_in = {(t, g): dram.tile([NC, 128, n, 128], BF, name=f"rs_in{t}{g}")
                 for t in range(2) for g, n in enumerate(CHUNKS[t])}
        rs_out = {(t, g): dram.tile([128, n, 128], BF, name=f"rs_out{t}{g}")
                  for t in range(2) for g, n in enumerate(CHUNKS[t])}
        with tc.tile_pool(name="s4", bufs=1) as s4, \
             tc.tile_pool(name="ps4", bufs=1, space="PSUM") as ps4:
            # hoist both halves' input loads so no later ring wait blocks them;
            # fp8 (gate) on sync ring, bf16 (up) on gpsimd ring
            h2g8s, h2gs = [], []
            for ts in range(2):
                h2g8 = s4.tile([128, DT, NC * 128], F8, tag="h2g8", bufs=2,
                               name=f"h2g8{ts}")
                h2g8s.append(h2g8)
                for j in range(NC):
                    nc.sync.dma_start(h2g8[:, :, j * 128:(j + 1) * 128],
                                      ag8_out[ts][j])
            for ts in range(2):
                h2g = s4.tile([128, DT, NC * 128], BF, tag="h2g", bufs=2,
                              name=f"h2g{ts}")
                h2gs.append(h2g)
                for j in range(NC):
                    nc.gpsimd.dma_start(h2g[:, :, j * 128:(j + 1) * 128],
                                        ag_out[ts][j])
            for ts in range(2):
                h2g8, h2g = h2g8s[ts], h2gs[ts]
                act_t = s4.tile([128, MT, 1024], BF, tag="act", bufs=1,
                                name=f"act{ts}")
                for mt in range(MT):
                    wg_t = s4.tile([128, DT, 128], F8, tag="wg", bufs=2)
                    wu_t = s4.tile([128, DT, 128], BF, tag="wu", bufs=2)
                    nc.scalar.dma_start(wg_t[:], wgTt8[mt])
                    nc.scalar.dma_start(wu_t[:], wuTt[mt])
                    g_ps = ps4.tile([128, 1024], F32, tag="g", name=f"g{ts}{mt}",
                                    bufs=2)
                    u_ps = ps4.tile([128, 1024], F32, tag="u", name=f"u{ts}{mt}",
                                    bufs=2)
                    for i in range(2):
                        for dtp in range(DTP):
                            nc.tensor.matmul(
                                g_ps[:, i * 512:(i + 1) * 512],
                                wg_t[:, 2 * dtp:2 * dtp + 2, :],
                                h2g8[:, 2 * dtp:2 * dtp + 2,
                                     i * 512:(i + 1) * 512],
                                start=(dtp == 0), stop=(dtp == DTP - 1),
                                perf_mode=DR)
                    for i in range(2):
                        for dt in range(DT):
                            nc.tensor.matmul(
                                u_ps[:, i * 512:(i + 1) * 512], wu_t[:, dt, :],
                                h2g[:, dt, i * 512:(i + 1) * 512],
                                start=(dt == 0), stop=(dt == DT - 1))
                    gs = s4.tile([128, 1024], BF, tag="gs", bufs=2)
                    nc.scalar.activation(gs[:], g_ps[:], AF.Sigmoid,
                                         scale=1.0 / (SW * SX))
                    nc.vector.tensor_tensor(out=act_t[:, mt, :], in0=u_ps[:],
                                            in1=gs[:], op=OP.mult)
                # down: contraction over mt; dn is j-major for contiguous RS DMA
                dt0 = 0
                for grp, nch in enumerate(CHUNKS[ts]):
                    dn = s4.tile([128, NC, 8, 128], BF, tag="dn", bufs=2,
                                 name=f"dn{ts}{grp}")
                    for k in range(nch):
                        dt = dt0 + k
                        wd_t = s4.tile([128, MT, 128], BF, tag="wd", bufs=3)
                        nc.scalar.dma_start(wd_t[:], wdTt[dt])
                        d_ps = ps4.tile([128, 1024], F32,
                                        tag=("g" if dt % 2 == 0 else "u"),
                                        name=f"d{ts}{dt}", bufs=2)
                        for mt in range(MT):
                            for i in range(2):
                                nc.tensor.matmul(
                                    d_ps[:, i * 512:(i + 1) * 512], wd_t[:, mt, :],
                                    act_t[:, mt, i * 512:(i + 1) * 512],
                                    start=(mt == 0), stop=(mt == MT - 1))
                        nc.vector.tensor_copy(dn[:, :, k, :], d_ps[:])
                    for j in range(NC):
                        nc.sync.dma_start(rs_in[ts, grp][j], dn[:, j, 0:nch, :])
                    nc.gpsimd.collective_compute(
                        "ReduceScatter", OP.add, ins=[rs_in[ts, grp][:]],
                        outs=[rs_out[ts, grp][:]], replica_groups=RG)
                    dt0 += nch
                # finalize each chunk as its RS lands (overlaps next work)
                dt0 = 0
                for grp, nch in enumerate(CHUNKS[ts]):
                    rsb = s4.tile([128, 8, 128], BF, tag="rsb", bufs=2)
                    nc.sync.dma_start(rsb[:, 0:nch, :], rs_out[ts, grp][:])
                    for k in range(nch):
                        dt = dt0 + k
                        fin = s4.tile([128, 128], F32, tag="fin", bufs=4)
                        nc.vector.tensor_tensor(
                            out=fin[:], in0=rsb[:, k, :],
                            in1=res1[dt][:, ts * 128:(ts + 1) * 128], op=OP.add)
                        nc.sync.dma_start(out_sh[dt * 128:(dt + 1) * 128,
                                                 ts * 128:(ts + 1) * 128], fin[:])
                    dt0 += nch

        res_ctx.close()

    nc.compile()
    return nc


_PROG = None


def _get_program():
    global _PROG
    if _PROG is None:
        _PROG = _build_program()
    return _PROG


def _prep_inputs(x, norm1_w, wq, wk, wv, wo, norm2_w, w_gate, w_up, w_down, cos, sin):
    x = np.asarray(x, dtype=np.float32)
    xT = np.ascontiguousarray(x.reshape(S, H).T)                       # [H, S]
    x8T = _q8(xT, SX)
    cosT = np.ascontiguousarray(np.asarray(cos, np.float32).T) / (SW * SX)
    sinT = np.ascontiguousarray(np.asarray(sin, np.float32).T) / (SW * SX)
    sinTs = sinT.copy()
    sinTs[0:HD // 2] = -sinTs[0:HD // 2]       # rotate_half sign for lo rows
    n1 = np.asarray(norm1_w, np.float32)
    n2 = np.asarray(norm2_w, np.float32)
    wq = np.asarray(wq, np.float32) * n1[None, :] / np.sqrt(np.float32(HD))
    wk = np.asarray(wk, np.float32) * n1[None, :]
    wv = np.asarray(wv, np.float32) * n1[None, :]
    wg = np.asarray(w_gate, np.float32) * n2[None, :]
    wu = np.asarray(w_up, np.float32) * n2[None, :]
    wo = np.asarray(wo, np.float32)
    wd = np.asarray(w_down, np.float32)

    woT8 = _q8(wo.T, SW)                                               # [e=H, d=H]
    # e-tile permute to head-major: slot et' = h*8 + j holds e-tile 2j+h
    woTt = woT8.reshape(DT, 128, DT, 128)
    perm = [2 * (e % 8) + (e // 8) for e in range(DT)]
    woTt8 = np.ascontiguousarray(
        woTt[perm].transpose(2, 1, 0, 3))                              # [dt, p, et', c]

    in_maps = []
    for c in range(NC):
        e0 = c * EH
        m0 = c * MSH
        wqkv = np.concatenate([wq[e0:e0 + EH, :], wk[e0:e0 + EH, :],
                               wv[e0:e0 + EH, :]], axis=0)             # [768, H]
        wqkvT8 = _q8(np.ascontiguousarray(wqkv.T), SW)                 # [H, 768]
        wgT8 = _q8(wg[m0:m0 + MSH, :].T, SW)                           # [H, MSH]
        wuT = wu[m0:m0 + MSH, :].T.astype(BF_NP)
        wdT = wd[:, m0:m0 + MSH].T.astype(BF_NP)                       # [MSH, H]
        in_maps.append({
            "x8T": x8T,
            "xTrs": np.ascontiguousarray(xT[:, c * SSH:(c + 1) * SSH]),
            "cosT": cosT, "sinTs": sinTs,
            "wqkvT8": wqkvT8,
            "woTt8": woTt8,
            "wgTt8": np.ascontiguousarray(
                wgT8.reshape(DT, 128, MT, 128).transpose(2, 1, 0, 3)),  # [mt,p,dt,c]
            "wuTt": np.ascontiguousarray(
                wuT.reshape(DT, 128, MT, 128).transpose(2, 1, 0, 3)),
            "wdTt": np.ascontiguousarray(
                wdT.reshape(MT, 128, DT, 128).transpose(2, 1, 0, 3)),  # [dt,p,mt,c]
        })
    return in_maps


def kernel(x, norm1_w, wq, wk, wv, wo, norm2_w, w_gate, w_up, w_down, cos, sin,
           _want_results=False):
    in_maps = _prep_inputs(x, norm1_w, wq, wk, wv, wo, norm2_w,
                           w_gate, w_up, w_down, cos, sin)
    prog = _get_program()
    res = run_bass_kernel_spmd(prog, in_maps, list(range(NC)))
    out = np.empty((B, S, H), dtype=np.float32)
    for c in range(NC):
        out[0, c * SSH:(c + 1) * SSH, :] = res.results[c]["out_sh"].T
    if _want_results:
        return out, res
    return out


# revision 13
# speedup vs baseline: 1.2029x; 1.0449x over previous
"""Trainium2 Bass kernel for nn_DeepseekLayer (dense transformer layer).

Sharding (8 cores): Megatron-style TP, fp8(e4m3)+bf16 datapath, fp32 PSUM.
  - qkv head-sharded (2 heads/core) over full S; x resident in SBUF as fp8
    (x*16); qkv/o/gate weights fp8 (*2^10), matmuls in DoubleRow perf mode
    (2 k-subtiles of 128 per instruction, ~1.5x bf16 throughput).
  - rmsnorm1 folded into rope tables / V-scale; rstd computed on device;
    x^2 computed on the (otherwise idle) ACT engine.
  - attention: transposed-softmax layout (scores [sk, sq]) in bf16 on
    512-wide sq chunks, exp(score-4) written as fp8 by ACT, softmax sums +
    probs@V as fp8 DoubleRow matmuls, fast-reciprocal normalize, per-head
    AllToAll (fp8, att*16).  qkv for head 1 is interleaved into head 0's
    attention instruction stream to keep PE and ACT both fed.
  - o_proj fp8 DoubleRow split by head: the head-0 half runs during the
    AllToAll(h1) flight, the head-1 half right after it lands.
  - rmsnorm2 + AllGather pipelined per token-half (fp8 h2*16 for gate,
    bf16 for up) with contiguous staging DMAs; MLP(ts0) overlaps the
    ts1 bridge.  gate fp8 DoubleRow, up/down bf16 (fp8 there would break
    the 2e-2 error budget) -> chunked bf16 ReduceScatter (j-major dn
    layout for contiguous staging, small last chunk) -> residual add.
"""
import numpy as np
from contextlib import ExitStack

import ml_dtypes
from concourse import bacc
import concourse.tile as tile
import concourse.mybir as mybir
from concourse.bass_utils import run_bass_kernel_spmd

F32 = mybir.dt.float32
BF = mybir.dt.bfloat16
F8 = mybir.dt.float8e4
AF = mybir.ActivationFunctionType
OP = mybir.AluOpType
DR = mybir.MatmulPerfMode.DoubleRow

H = 2048          # hidden
NH = 16           # heads
HD = 128          # head dim
MLP = 8192
S = 2048          # sequence
B = 1
EPS = 1e-6
NC = 8            # cores
HPC = NH // NC    # heads per core = 2
EH = HPC * HD     # qkv out dims per core = 256
MSH = MLP // NC   # mlp dims per core = 1024
SSH = S // NC     # tokens per shard = 256
RG = [list(range(NC))]
DT = H // 128     # 16 d-tiles
DTP = DT // 2     # 8 d-tile pairs
MT = MSH // 128   # 8 m-tiles per core
BF_NP = ml_dtypes.bfloat16
F8_NP = ml_dtypes.float8_e4m3   # TRN fp8e4: max +-240

SX = 16.0         # x / activation fp8 scale
SW = 1024.0       # weight fp8 scale (2^10)
ECONST = 4.0      # exp(score - ECONST); cancels in softmax


def _q8(a, scale):
    return np.clip(np.asarray(a, np.float32) * scale, -240.0, 240.0).astype(F8_NP)


def _merge(a_thunks, b_thunks):
    """Emit a-thunks in order with b-thunks proportionally interspersed."""
    na, nb = len(a_thunks), len(b_thunks)
    ib = 0
    for i, a in enumerate(a_thunks):
        a()
        while ib < nb and ib * na < (i + 1) * nb:
            b_thunks[ib]()
            ib += 1
    while ib < nb:
        b_thunks[ib]()
        ib += 1


def _build_program():
    nc = bacc.Bacc(trn_type="TRN2", target_bir_lowering=False, debug=False,
                   num_devices=NC)

    def inp(name, shape, dt):
        return nc.dram_tensor(name, shape, dt, kind="ExternalInput").ap()

    x8T = inp("x8T", [H, S], F8)                # x.T * 16, fp8
    xTrs = inp("xTrs", [H, SSH], F32)           # this core's token-shard, f32
    cosT = inp("cosT", [HD, S], F32)            # cos.T / (SW*SX)
    sinTs = inp("sinTs", [HD, S], F32)          # sin.T / (SW*SX), rows 0:63 neg
    wqkvT8 = inp("wqkvT8", [H, 6 * 128], F8)    # cols: q0,q1,k0,k1,v0,v1 (*SW)
    woTt8 = inp("woTt8", [DT, 128, DT, 128], F8)  # wo.T [dt, p, et', c] head-major
    wgTt8 = inp("wgTt8", [MT, 128, DT, 128], F8)  # (wg*n2).T shard *SW
    wuTt = inp("wuTt", [MT, 128, DT, 128], BF)
    wdTt = inp("wdTt", [DT, 128, MT, 128], BF)
    out_sh = nc.dram_tensor("out_sh", [H, SSH], F32, kind="ExternalOutput").ap()

    with tile.TileContext(nc) as tc, ExitStack() as top:
        dram = top.enter_context(tc.tile_pool(name="dram", bufs=1, space="DRAM"))
        per = top.enter_context(tc.tile_pool(name="per", bufs=1))
        ones_f = per.tile([128, 1], F32)
        nc.gpsimd.memset(ones_f[:], 1.0)
        ones_b = per.tile([128, 1], BF)
        nc.vector.tensor_copy(ones_b[:], ones_f[:])
        ones8 = per.tile([128, 2, 16], F8)
        nc.gpsimd.memset(ones8[:], 1.0)
        eps1 = per.tile([1, 1], F32)
        nc.gpsimd.memset(eps1[:], EPS)
        eC = per.tile([128, 1], F32)
        nc.gpsimd.memset(eC[:], -ECONST)
        from concourse.masks import make_identity
        ident_f = per.tile([128, 128], F32)
        make_identity(nc, ident_f[:])
        ident_b = per.tile([128, 128], BF)
        nc.vector.tensor_copy(ident_b[:], ident_f[:])

        # ---- persistent SBUF: fp32 residual + o accumulator ----
        res_ctx = ExitStack()
        res_pool = res_ctx.enter_context(tc.tile_pool(name="res", bufs=1))
        res1 = [res_pool.tile([128, SSH], F32, name=f"res1_{dt}") for dt in range(DT)]

        qk_ctx = ExitStack()
        qk = qk_ctx.enter_context(tc.tile_pool(name="qk", bufs=1))
        qr = [qk.tile([128, S], BF, name=f"qr{h}") for h in range(HPC)]
        kr = [qk.tile([128, S], BF, name=f"kr{h}") for h in range(HPC)]
        V_sb = qk.tile([128, S // 128, EH], F8, name="V_sb")
        att = [qk.tile([128, S], F8, name=f"att{h}") for h in range(HPC)]
        wo_sb = qk.tile([128, DT, DT, 128], F8, name="wo_sb")   # [p, dt, et', c]
        attg = [qk.tile([128, DTP, SSH], F8, name=f"attg{h}")
                for h in range(HPC)]                            # slot j of head h

        a2a_in = [dram.tile([NC, 128, SSH], F8, name=f"a2a_in{h}") for h in range(HPC)]
        a2a_out = [dram.tile([NC, 128, SSH], F8, name=f"a2a_out{h}") for h in range(HPC)]

        # x / tables / qkv weights: live through P1+P2
        x_ctx = ExitStack()
        xp = x_ctx.enter_context(tc.tile_pool(name="xp", bufs=1))
        x_sb = xp.tile([128, DT, S], F8, name="x_sb")
        wqkv_sb = xp.tile([128, DT, 6 * 128], F8, name="wqkv_sb")
        cs_c = xp.tile([HD, S], F32, name="cs_c")     # cos * rstd / (SW*SX)
        cs_s = xp.tile([HD, S], F32, name="cs_s")
        rstd_bc = xp.tile([128, S], F32, name="rstd_bc")
        xr = xp.tile([128, DT, SSH], F32, name="xr")
        o_acc = xp.tile([128, DT, SSH], F32, name="o_acc")

        def rope(pool, dst, ps, c0):
            mc = pool.tile([128, 1024], F32, tag="mc", bufs=1)
            msw = pool.tile([128, 1024], F32, tag="msw", bufs=1)
            nc.vector.tensor_tensor(out=mc[:], in0=ps[:],
                                    in1=cs_c[:, c0:c0 + 1024], op=OP.mult)
            nc.vector.tensor_tensor(out=msw[0:64, :], in0=ps[64:128, :],
                                    in1=cs_s[0:64, c0:c0 + 1024], op=OP.mult)
            nc.vector.tensor_tensor(out=msw[64:128, :], in0=ps[0:64, :],
                                    in1=cs_s[64:128, c0:c0 + 1024], op=OP.mult)
            nc.vector.tensor_tensor(out=dst[:, c0:c0 + 1024], in0=mc[:],
                                    in1=msw[:], op=OP.add)

        def qkv_units(pool, pspool, h, tag, psbufs=2):
            """Thunk list: qkv projections for head h, 4-MM units."""
            units = []
            for kind in ("q", "k", "v"):
                oc = {"q": 0, "k": 2, "v": 4}[kind] + h
                for half in range(2):
                    c0 = half * 1024
                    ps = pspool.tile([128, 1024], F32, tag=tag,
                                     name=f"{kind}{h}_{half}", bufs=psbufs)

                    def mm(dtp0, ps=ps, oc=oc, c0=c0):
                        for dtp in range(dtp0, dtp0 + 2):
                            for i in range(2):
                                nc.tensor.matmul(
                                    ps[:, i * 512:(i + 1) * 512],
                                    wqkv_sb[:, 2 * dtp:2 * dtp + 2,
                                            oc * 128:(oc + 1) * 128],
                                    x_sb[:, 2 * dtp:2 * dtp + 2,
                                         c0 + i * 512:c0 + (i + 1) * 512],
                                    start=(dtp == 0), stop=(dtp == DTP - 1),
                                    perf_mode=DR)
                    for dtp0 in range(0, DTP, 2):
                        units.append(lambda dtp0=dtp0, mm=mm: mm(dtp0))

                    def fin(ps=ps, kind=kind, c0=c0, half=half, h=h,
                            psbufs=psbufs):
                        if kind == "q":
                            rope(pool, qr[h], ps[:], c0)
                        elif kind == "k":
                            rope(pool, kr[h], ps[:], c0)
                        else:
                            vsc = pool.tile([128, 1024], BF, tag="vsc", bufs=2)
                            nc.vector.tensor_tensor(out=vsc[:], in0=ps[:],
                                                    in1=rstd_bc[:, c0:c0 + 1024],
                                                    op=OP.mult)
                            for j in range(8):
                                kt = half * 8 + j
                                tr = pspool.tile([128, 128], BF, tag=tag,
                                                 name=f"tr{h}{kt}", bufs=psbufs)
                                nc.tensor.transpose(tr[:],
                                                    vsc[:, j * 128:(j + 1) * 128],
                                                    ident_b[:])
                                nc.vector.tensor_scalar(
                                    V_sb[:, kt, h * 128:(h + 1) * 128], tr[:],
                                    16.0 / (SW * SX), None, OP.mult)
                    units.append(fin)
            return units

        # ============ P1: x load + rmsnorm1 stats + qkv head0 ============
        p1_ctx = ExitStack()
        s1 = p1_ctx.enter_context(tc.tile_pool(name="s1", bufs=1))
        ps1 = p1_ctx.enter_context(tc.tile_pool(name="ps1", bufs=1, space="PSUM"))

        ss_ps = ps1.tile([1, S], F32, tag="stat", name="ss_ps", bufs=1)
        for dt in range(DT):
            nc.sync.dma_start(x_sb[:, dt, :], x8T[dt * 128:(dt + 1) * 128, :])
            nc.sync.dma_start(wqkv_sb[:, dt, :], wqkvT8[dt * 128:(dt + 1) * 128, :])
            if dt == 2:
                nc.scalar.dma_start(cs_c[:], cosT)
                nc.scalar.dma_start(cs_s[:], sinTs)
            for i in range(2):
                sq = s1.tile([128, 1024], BF, tag="sq", bufs=4)
                if dt % 2 == 0:
                    nc.scalar.activation(sq[:], x_sb[:, dt, i * 1024:(i + 1) * 1024],
                                         AF.Square)
                else:
                    nc.vector.tensor_tensor(out=sq[:],
                                            in0=x_sb[:, dt, i * 1024:(i + 1) * 1024],
                                            in1=x_sb[:, dt, i * 1024:(i + 1) * 1024],
                                            op=OP.mult)
                for j in range(2):
                    c = i * 1024 + j * 512
                    nc.tensor.matmul(ss_ps[:, c:c + 512], ones_b[:],
                                     sq[:, j * 512:(j + 1) * 512],
                                     start=(dt == 0), stop=(dt == DT - 1))
        # sq = (16x)^2 -> rstd = 1/sqrt(ss/(256 H) + eps)
        for i in range(4):
            sdc = s1.tile([1, 512], F32, tag="sdc", bufs=2)
            nc.scalar.activation(sdc[:], ss_ps[:, i * 512:(i + 1) * 512], AF.Sqrt,
                                 bias=eps1[:], scale=1.0 / (H * SX * SX))
            rsc = s1.tile([1, 512], F32, tag="rsc", bufs=2)
            nc.vector.reciprocal_approx_fast(out=rsc[:], in_=sdc[:])
            nc.gpsimd.partition_broadcast(rstd_bc[:, i * 512:(i + 1) * 512], rsc[:],
                                          channels=128)
        nc.vector.tensor_tensor(out=cs_c[:], in0=cs_c[:], in1=rstd_bc[:], op=OP.mult)
        nc.vector.tensor_tensor(out=cs_s[:], in0=cs_s[:], in1=rstd_bc[:], op=OP.mult)

        for u in qkv_units(s1, ps1, 0, "qk"):
            u()
        p1_ctx.close()

        # ============ P2: attn h0 (+qkv h1 interleaved), o split, a2a ============
        p2_ctx = ExitStack()
        s2 = p2_ctx.enter_context(tc.tile_pool(name="s2", bufs=1))
        ps2 = p2_ctx.enter_context(tc.tile_pool(name="ps2", bufs=1, space="PSUM"))

        def attn_units(h):
            """Thunk list for attention head h on 512-wide sq chunks."""
            units = []
            for sc in range(4):
                c0 = sc * 512
                box = {}

                def open_chunk(box=box, h=h, sc=sc):
                    box["av"] = ps2.tile([128, 512], F32, tag="av",
                                         name=f"av{h}{sc}", bufs=1)
                    box["sm"] = ps2.tile([1, 512], F32, tag="sm",
                                         name=f"sm{h}{sc}", bufs=1)
                units.append(open_chunk)

                def unit(ktp, box=box, h=h, c0=c0):
                    e8 = s2.tile([128, 2, 512], F8, tag="e", bufs=5)
                    for sub in range(2):
                        kt = 2 * ktp + sub
                        st = ps2.tile([128, 512], F32, tag="st",
                                      name=f"st{h}", bufs=4)
                        nc.tensor.matmul(st[:],
                                         kr[h][:, kt * 128:(kt + 1) * 128],
                                         qr[h][:, c0:c0 + 512],
                                         start=True, stop=True)
                        nc.scalar.activation(e8[:, sub, :], st[:], AF.Exp,
                                             bias=eC[:])
                    nc.tensor.matmul(box["sm"][:], ones8[:, :, 0:1], e8[:],
                                     start=(ktp == 0), stop=(ktp == DTP - 1),
                                     perf_mode=DR)
                    nc.tensor.matmul(box["av"][:],
                                     V_sb[:, 2 * ktp:2 * ktp + 2,
                                          h * 128:(h + 1) * 128],
                                     e8[:],
                                     start=(ktp == 0), stop=(ktp == DTP - 1),
                                     perf_mode=DR)
                for ktp in range(DTP):
                    units.append(lambda ktp=ktp, unit=unit: unit(ktp))

                def close_chunk(box=box, h=h, c0=c0):
                    rs = s2.tile([1, 512], F32, tag="rs", bufs=2)
                    nc.vector.reciprocal_approx_fast(out=rs[:], in_=box["sm"][:])
                    bc = s2.tile([128, 512], F32, tag="bc", bufs=2)
                    nc.gpsimd.partition_broadcast(bc[:], rs[:], channels=128)
                    nc.vector.tensor_tensor(out=att[h][:, c0:c0 + 512],
                                            in0=box["av"][:], in1=bc[:], op=OP.mult)
                units.append(close_chunk)
            return units

        def fire_a2a(h):
            for j in range(NC):
                nc.sync.dma_start(a2a_in[h][j], att[h][:, j * SSH:(j + 1) * SSH])
            nc.gpsimd.collective_compute("AllToAll", OP.bypass,
                                         ins=[a2a_in[h][:]], outs=[a2a_out[h][:]],
                                         replica_groups=RG)

        # prefetch wo (4MB fp8) + residual: no deps, DMA ring runs them early
        for dt in range(DT):
            nc.sync.dma_start(wo_sb[:, dt], woTt8[dt])
            nc.sync.dma_start(xr[:, dt, :], xTrs[dt * 128:(dt + 1) * 128, :])
        # attn h0 with qkv h1 interleaved into its instruction stream
        _merge(attn_units(0), qkv_units(s2, ps2, 1, "qk2", psbufs=1))
        fire_a2a(0)
        for j in range(NC):
            nc.sync.dma_start(attg[0][:, j, :], a2a_out[0][j])

        for u in attn_units(1):
            u()

        # o_proj head-0 half: runs while AllToAll(h1) is in flight.
        # o_ps = o_true * SW * 16; o_acc = o_h0 / (SW*16) + xr
        for dtq in range(DTP):
            o_ps = ps2.tile([128, 512], F32, tag="st", name=f"o0_{dtq}", bufs=4)
            for dtl in range(2):
                dt = 2 * dtq + dtl
                for ep in range(4):
                    nc.tensor.matmul(
                        o_ps[:, dtl * 256:(dtl + 1) * 256],
                        wo_sb[:, dt, 2 * ep:2 * ep + 2, :],
                        attg[0][:, 2 * ep:2 * ep + 2, :],
                        start=(ep == 0), stop=(ep == 3), perf_mode=DR)
            for dtl in range(2):
                dt = 2 * dtq + dtl
                nc.vector.tensor_scalar(o_acc[:, dt, :],
                                        o_ps[:, dtl * 256:(dtl + 1) * 256],
                                        1.0 / (SW * 16.0), None, OP.mult)
                nc.vector.tensor_tensor(out=o_acc[:, dt, :], in0=o_acc[:, dt, :],
                                        in1=xr[:, dt, :], op=OP.add)
        fire_a2a(1)
        for j in range(NC):
            nc.sync.dma_start(attg[1][:, j, :], a2a_out[1][j])
        # o_proj head-1 half (waits on AllToAll(h1)) + residual
        for dtq in range(DTP):
            o_ps = ps2.tile([128, 512], F32, tag="st", name=f"o1_{dtq}", bufs=4)
            for dtl in range(2):
                dt = 2 * dtq + dtl
                for ep in range(4):
                    nc.tensor.matmul(
                        o_ps[:, dtl * 256:(dtl + 1) * 256],
                        wo_sb[:, dt, 2 * (ep + 4):2 * (ep + 4) + 2, :],
                        attg[1][:, 2 * ep:2 * ep + 2, :],
                        start=(ep == 0), stop=(ep == 3), perf_mode=DR)
            for dtl in range(2):
                dt = 2 * dtq + dtl
                obf = s2.tile([128, SSH], F32, tag="obf", bufs=2)
                nc.vector.tensor_scalar(obf[:], o_ps[:, dtl * 256:(dtl + 1) * 256],
                                        1.0 / (SW * 16.0), None, OP.mult)
                nc.vector.tensor_tensor(out=res1[dt][:], in0=obf[:],
                                        in1=o_acc[:, dt, :], op=OP.add)
        p2_ctx.close()
        x_ctx.close()

        # ============ P3: rmsnorm2 + AG, pipelined per token-half ============
        ag8_in = [dram.tile([128, DT, 128], F8, name=f"ag8_in{t}") for t in range(2)]
        ag8_out = [dram.tile([NC, 128, DT, 128], F8, addr_space="Shared",
                             name=f"ag8_out{t}") for t in range(2)]
        ag_in = [dram.tile([128, DT, 128], BF, name=f"ag_in{t}") for t in range(2)]
        ag_out = [dram.tile([NC, 128, DT, 128], BF, addr_space="Shared",
                            name=f"ag_out{t}") for t in range(2)]
        with tc.tile_pool(name="s3", bufs=1) as s3, \
             tc.tile_pool(name="ps3", bufs=1, space="PSUM") as ps3:
            for ts in range(2):
                tsl = slice(ts * 128, (ts + 1) * 128)
                ss2_ps = ps3.tile([1, 128], F32, tag="ss2", name=f"ss2_{ts}",
                                  bufs=2)
                for dt in range(DT):
                    sq2 = s3.tile([128, 128], BF, tag="sq2", bufs=3)
                    nc.vector.tensor_tensor(out=sq2[:], in0=res1[dt][:, tsl],
                                            in1=res1[dt][:, tsl], op=OP.mult)
                    nc.tensor.matmul(ss2_ps[:], ones_b[:], sq2[:],
                                     start=(dt == 0), stop=(dt == DT - 1))
                sd2 = s3.tile([1, 128], F32, tag="sd2", bufs=2)
                nc.scalar.activation(sd2[:], ss2_ps[:], AF.Sqrt, bias=eps1[:],
                                     scale=1.0 / H)
                rstd2 = s3.tile([1, 128], F32, tag="rstd2", bufs=2)
                nc.vector.reciprocal_approx_fast(out=rstd2[:], in_=sd2[:])
                rstd2_bc = s3.tile([128, 128], F32, tag="r2bc", bufs=2)
                nc.gpsimd.partition_broadcast(rstd2_bc[:], rstd2[:], channels=128)
                h2t = s3.tile([128, DT, 128], BF, tag="h2t", bufs=2,
                              name=f"h2t{ts}")
                h8t = s3.tile([128, DT, 128], F8, tag="h8t", bufs=2,
                              name=f"h8t{ts}")
                for dt in range(DT):
                    nc.vector.tensor_tensor(out=h2t[:, dt, :], in0=res1[dt][:, tsl],
                                            in1=rstd2_bc[:], op=OP.mult)
                    nc.vector.tensor_scalar(h8t[:, dt, :], h2t[:, dt, :], SX,
                                            None, OP.mult)
                # contiguous staging DMAs + AGs; fp8 first (gate needs it first)
                nc.sync.dma_start(ag8_in[ts][:], h8t[:])
                nc.gpsimd.collective_compute("AllGather", OP.bypass,
                                             ins=[ag8_in[ts][:]],
                                             outs=[ag8_out[ts][:]],
                                             replica_groups=RG)
                nc.sync.dma_start(ag_in[ts][:], h2t[:])
                nc.gpsimd.collective_compute("AllGather", OP.bypass,
                                             ins=[ag_in[ts][:]], outs=[ag_out[ts][:]],
                                             replica_groups=RG)
        qk_ctx.close()

        # ============ P4: MLP per token-half, RS chunked (last one small) ====
        CHUNKS = {0: [8, 8], 1: [8, 8]}
        rs_in = {(t, g): dram.tile([NC, 128, n, 128], BF, name=f"rs_in{t}{g}")
                 for t in range(2) for g, n in enumerate(CHUNKS[t])}
        rs_out = {(t, g): dram.tile([128, n, 128], BF, name=f"rs_out{t}{g}")
                  for t in range(2) for g, n in enumerate(CHUNKS[t])}
        with tc.tile_pool(name="s4", bufs=1) as s4, \
             tc.tile_pool(name="ps4", bufs=1, space="PSUM") as ps4:
            # hoist both halves' input loads so no later ring wait blocks them;
            # fp8 (gate) on sync ring, bf16 (up) on gpsimd ring
            rings = [nc.sync, nc.gpsimd, nc.scalar]
            h2g8s, h2gs = [], []
            for ts in range(2):
                h2g8s.append(s4.tile([128, DT, NC * 128], F8, tag="h2g8", bufs=2,
                                     name=f"h2g8{ts}"))
                h2gs.append(s4.tile([128, DT, NC * 128], BF, tag="h2g", bufs=2,
                                    name=f"h2g{ts}"))
            for ts in range(2):
                for j in range(NC):
                    rings[j % 3].dma_start(h2g8s[ts][:, :, j * 128:(j + 1) * 128],
                                           ag8_out[ts][j])
                for j in range(NC):
                    rings[j % 3].dma_start(h2gs[ts][:, :, j * 128:(j + 1) * 128],
                                           ag_out[ts][j])
            for ts in range(2):
                h2g8, h2g = h2g8s[ts], h2gs[ts]
                act_t = s4.tile([128, MT, 1024], BF, tag="act", bufs=1,
                                name=f"act{ts}")
                for mt in range(MT):
                    wg_t = s4.tile([128, DT, 128], F8, tag="wg", bufs=2)
                    wu_t = s4.tile([128, DT, 128], BF, tag="wu", bufs=2)
                    nc.scalar.dma_start(wg_t[:], wgTt8[mt])
                    nc.scalar.dma_start(wu_t[:], wuTt[mt])
                    g_ps = ps4.tile([128, 1024], F32, tag="g", name=f"g{ts}{mt}",
                                    bufs=2)
                    u_ps = ps4.tile([128, 1024], F32, tag="u", name=f"u{ts}{mt}",
                                    bufs=2)
                    for i in range(2):
                        for dtp in range(DTP):
                            nc.tensor.matmul(
                                g_ps[:, i * 512:(i + 1) * 512],
                                wg_t[:, 2 * dtp:2 * dtp + 2, :],
                                h2g8[:, 2 * dtp:2 * dtp + 2,
                                     i * 512:(i + 1) * 512],
                                start=(dtp == 0), stop=(dtp == DTP - 1),
                                perf_mode=DR)
                    for i in range(2):
                        for dt in range(DT):
                            nc.tensor.matmul(
                                u_ps[:, i * 512:(i + 1) * 512], wu_t[:, dt, :],
                                h2g[:, dt, i * 512:(i + 1) * 512],
                                start=(dt == 0), stop=(dt == DT - 1))
                    gs = s4.tile([128, 1024], BF, tag="gs", bufs=2)
                    nc.scalar.activation(gs[:], g_ps[:], AF.Sigmoid,
                                         scale=1.0 / (SW * SX))
                    nc.vector.tensor_tensor(out=act_t[:, mt, :], in0=u_ps[:],
                                            in1=gs[:], op=OP.mult)
                # down: contraction over mt; dn is j-major for contiguous RS DMA
                dt0 = 0
                for grp, nch in enumerate(CHUNKS[ts]):
                    dn = s4.tile([128, NC, 8, 128], BF, tag="dn", bufs=2,
                                 name=f"dn{ts}{grp}")
                    for k in range(nch):
                        dt = dt0 + k
                        wd_t = s4.tile([128, MT, 128], BF, tag="wd", bufs=3)
                        nc.scalar.dma_start(wd_t[:], wdTt[dt])
                        d_ps = ps4.tile([128, 1024], F32,
                                        tag=("g" if dt % 2 == 0 else "u"),
                                        name=f"d{ts}{dt}", bufs=2)
                        for mt in range(MT):
                            for i in range(2):
                                nc.tensor.matmul(
                                    d_ps[:, i * 512:(i + 1) * 512], wd_t[:, mt, :],
                                    act_t[:, mt, i * 512:(i + 1) * 512],
                                    start=(mt == 0), stop=(mt == MT - 1))
                        nc.vector.tensor_copy(dn[:, :, k, :], d_ps[:])
                    for j in range(NC):
                        nc.sync.dma_start(rs_in[ts, grp][j], dn[:, j, 0:nch, :])
                    nc.gpsimd.collective_compute(
                        "ReduceScatter", OP.add, ins=[rs_in[ts, grp][:]],
                        outs=[rs_out[ts, grp][:]], replica_groups=RG)
                    dt0 += nch
                # finalize each chunk as its RS lands (overlaps next work)
                dt0 = 0
                for grp, nch in enumerate(CHUNKS[ts]):
                    rsb = s4.tile([128, 8, 128], BF, tag="rsb", bufs=2)
                    nc.sync.dma_start(rsb[:, 0:nch, :], rs_out[ts, grp][:])
                    for k in range(nch):
                        dt = dt0 + k
                        fin = s4.tile([128, 128], F32, tag="fin", bufs=4)
                        nc.vector.tensor_tensor(
                            out=fin[:], in0=rsb[:, k, :],
                            in1=res1[dt][:, ts * 128:(ts + 1) * 128], op=OP.add)
                        nc.sync.dma_start(out_sh[dt * 128:(dt + 1) * 128,
                                                 ts * 128:(ts + 1) * 128], fin[:])
                    dt0 += nch

        res_ctx.close()

    nc.compile()
    return nc


_PROG = None


def _get_program():
    global _PROG
    if _PROG is None:
        _PROG = _build_program()
    return _PROG


def _prep_inputs(x, norm1_w, wq, wk, wv, wo, norm2_w, w_gate, w_up, w_down, cos, sin):
    x = np.asarray(x, dtype=np.float32)
    xT = np.ascontiguousarray(x.reshape(S, H).T)                       # [H, S]
    x8T = _q8(xT, SX)
    cosT = np.ascontiguousarray(np.asarray(cos, np.float32).T) / (SW * SX)
    sinT = np.ascontiguousarray(np.asarray(sin, np.float32).T) / (SW * SX)
    sinTs = sinT.copy()
    sinTs[0:HD // 2] = -sinTs[0:HD // 2]       # rotate_half sign for lo rows
    n1 = np.asarray(norm1_w, np.float32)
    n2 = np.asarray(norm2_w, np.float32)
    wq = np.asarray(wq, np.float32) * n1[None, :] / np.sqrt(np.float32(HD))
    wk = np.asarray(wk, np.float32) * n1[None, :]
    wv = np.asarray(wv, np.float32) * n1[None, :]
    wg = np.asarray(w_gate, np.float32) * n2[None, :]
    wu = np.asarray(w_up, np.float32) * n2[None, :]
    wo = np.asarray(wo, np.float32)
    wd = np.asarray(w_down, np.float32)

    woT8 = _q8(wo.T, SW)                                               # [e=H, d=H]
    # e-tile permute to head-major: slot et' = h*8 + j holds e-tile 2j+h
    woTt = woT8.reshape(DT, 128, DT, 128)
    perm = [2 * (e % 8) + (e // 8) for e in range(DT)]
    woTt8 = np.ascontiguousarray(
        woTt[perm].transpose(2, 1, 0, 3))                              # [dt, p, et', c]

    in_maps = []
    for c in range(NC):
        e0 = c * EH
        m0 = c * MSH
        wqkv = np.concatenate([wq[e0:e0 + EH, :], wk[e0:e0 + EH, :],
                               wv[e0:e0 + EH, :]], axis=0)             # [768, H]
        wqkvT8 = _q8(np.ascontiguousarray(wqkv.T), SW)                 # [H, 768]
        wgT8 = _q8(wg[m0:m0 + MSH, :].T, SW)                           # [H, MSH]
        wuT = wu[m0:m0 + MSH, :].T.astype(BF_NP)
        wdT = wd[:, m0:m0 + MSH].T.astype(BF_NP)                       # [MSH, H]
        in_maps.append({
            "x8T": x8T,
            "xTrs": np.ascontiguousarray(xT[:, c * SSH:(c + 1) * SSH]),
            "cosT": cosT, "sinTs": sinTs,
            "wqkvT8": wqkvT8,
            "woTt8": woTt8,
            "wgTt8": np.ascontiguousarray(
                wgT8.reshape(DT, 128, MT, 128).transpose(2, 1, 0, 3)),  # [mt,p,dt,c]
            "wuTt": np.ascontiguousarray(
                wuT.reshape(DT, 128, MT, 128).transpose(2, 1, 0, 3)),
            "wdTt": np.ascontiguousarray(
                wdT.reshape(MT, 128, DT, 128).transpose(2, 1, 0, 3)),  # [dt,p,mt,c]
        })
    return in_maps


def kernel(x, norm1_w, wq, wk, wv, wo, norm2_w, w_gate, w_up, w_down, cos, sin,
           _want_results=False):
    in_maps = _prep_inputs(x, norm1_w, wq, wk, wv, wo, norm2_w,
                           w_gate, w_up, w_down, cos, sin)
    prog = _get_program()
    res = run_bass_kernel_spmd(prog, in_maps, list(range(NC)))
    out = np.empty((B, S, H), dtype=np.float32)
    for c in range(NC):
        out[0, c * SSH:(c + 1) * SSH, :] = res.results[c]["out_sh"].T
    if _want_results:
        return out, res
    return out
